# revision 20
# baseline (speedup 1.0000x reference)
"""Causal local-window (W=128) attention block + FFN, distributed over 8 TRN2
NeuronCores with ZERO collectives.

Sharding: (B=2, L=2048) tokens are split into 8 contiguous segments of 512
tokens (4 per batch element). Each core receives its 512 owned tokens plus a
128-token left halo (zero-padded for the first segment of each batch) and
recomputes the halo's K/V locally — the sliding window (j in [i-128, i]) never
crosses more than 128 tokens back, so no cross-core communication is needed.

Key implementation choices (v2):
  - fp8(e4m3) weights + activations with MatmulPerfMode.DoubleRow for the
    QKV projections and the attention out-projection (2 K-chunks contracted
    per pass). FFN + attention internals stay bf16 (fp8 there would break
    the 2e-2 error budget; measured headroom: attn-side fp8 = 1.4e-2).
  - The additive sliding-window mask is applied on the PE: an identity
    matmul writes the mask into PSUM (start=True) and the score matmuls
    accumulate on top — no f32 DVE bias-add in the softmax path.
  - exp() runs on the Scalar engine straight from PSUM; row sums + the
    1/sum normalization run on DVE in bf16 (2x mode); the transposed-
    probability PSUM eviction runs on GpSimd; ctx eviction on Scalar.
    The attention inner loop is software-pipelined (scores run 2
    iterations ahead of transpose+ctx) so the PE never idles — TRN2 drops
    the PE clock 2x for ~3us after any idle gap.
  - LayerNorm rstd = exp(-0.5*ln(var+eps)): Ln, Exp, Identity and Copy all
    live in one activation table, so the Scalar engine never reloads
    tables inside the attention phase (a reload is 1.3us).
"""

import os
import numpy as np
import ml_dtypes

import concourse.bass as bass
import concourse.mybir as mybir
import concourse.tile as tile
from concourse.masks import make_identity
from bass_rust import ScopedClock

# ---------------------------------------------------------------------------
# Workarounds for the walrus build in this container, which accepts at most
# ONE sync-wait and ONE sync-update per instruction. Tile attaches one wait
# per out-of-date producer clock and one update per consumer engine, so any
# nontrivial Tile kernel violates this. Fix by splitting the extras onto
# standalone InstEventSemaphore instructions on the same engine: waits go
# immediately BEFORE the instruction, updates immediately AFTER (each engine
# executes its stream in order, so semantics are preserved).
_split_counter = [0]


def _split_multi_sync(nc):
    for f in nc.m.functions:
        for bb in f.blocks:
            il = list(bb.instructions)
            new = []
            changed = False
            for inst in il:
                si = inst.sync_info
                waits = list(si.on_wait) if si and si.on_wait else []
                upds = list(si.on_update) if si and si.on_update else []
                if len(waits) > 1:
                    changed = True
                    for w in waits[:-1]:
                        _split_counter[0] += 1
                        new.append(mybir.InstEventSemaphore(
                            name=f"I-wsplit-{_split_counter[0]}",
                            engine=inst.engine, ins=[], outs=[],
                            sync_info=mybir.SyncInfo(on_wait=[w], on_update=[]),
                        ))
                    si.on_wait = [waits[-1]]
                new.append(inst)
                if len(upds) > 1:
                    changed = True
                    si.on_update = [upds[0]]
                    for u in upds[1:]:
                        _split_counter[0] += 1
                        new.append(mybir.InstEventSemaphore(
                            name=f"I-usplit-{_split_counter[0]}",
                            engine=inst.engine, ins=[], outs=[],
                            sync_info=mybir.SyncInfo(on_wait=[], on_update=[u]),
                        ))
            if changed:
                bb.instructions = new


def _patched_drain_and_barrier(self, tick_clock, wait_clock):
    # Tile's kernel-tail drain carries one wait per logical processor; split
    # them into standalone single-wait SP instructions instead.
    nc = self.nc
    drain_inst = nc.sync.drain()
    wait_clock.add_sem_waits(drain_inst.ins, ScopedClock({None: tick_clock.global_clock}))
    si = drain_inst.ins.sync_info
    waits = list(si.on_wait or [])
    if len(waits) > 1:
        si.on_wait = []
        handles = {}
        for s in self.sems.allocated().values():
            nm = getattr(s, 'ant_name', None) or getattr(s, 'name', None)
            handles[nm] = s
        for w in waits:
            assert w.wait_mode == 'sem-ge-imm', w
            nc.sync.wait_ge(handles[w.ant_name], w.wait_value)
    nc.all_engine_barrier()
    assert self.sems is not None
    popped = nc._tile_sem_poison_stack.pop()
    assert popped is self._sem_poison
    nc.clear_and_free_semaphores(list(self.sems.allocated().values()))
    nc.all_engine_barrier()


tile.TileContext._drain_and_barrier = _patched_drain_and_barrier

F32 = mybir.dt.float32
BF16 = mybir.dt.bfloat16
AF = mybir.ActivationFunctionType
AX = mybir.AxisListType

# debug toggles (read at build time)
_USE_FP8 = os.environ.get("K_FP8", "1") == "1"
_USE_DR = os.environ.get("K_DR", "1") == "1" and _USE_FP8
FP8 = mybir.dt.float8e4 if _USE_FP8 else mybir.dt.bfloat16
DR = mybir.MatmulPerfMode.DoubleRow if _USE_DR else None

B, L, D = 2, 2048, 1024
NH, DH = 16, 64
DFF = 4096
WIN = 128
SEG = 512          # owned tokens per core
HALO = 128
T = SEG + HALO     # 640 local tokens
NT = T // 128      # 5 local token tiles
NSEG = 8           # cores
NEG = -1.0e30
LN_EPS = 1e-5

_CACHED = {}


def _build(split=True):
    nc = bass.Bass()
    x_ext = nc.declare_dram_parameter("x", [T, D], F32, isOutput=False)
    wq_ext = nc.declare_dram_parameter("wq", [D, D], FP8, isOutput=False)
    wk_ext = nc.declare_dram_parameter("wk", [D, D], FP8, isOutput=False)
    wv_ext = nc.declare_dram_parameter("wv", [D, D], FP8, isOutput=False)
    wo_ext = nc.declare_dram_parameter("wo", [D, D], FP8, isOutput=False)
    w1_ext = nc.declare_dram_parameter("w1", [D, DFF], BF16, isOutput=False)
    w2_ext = nc.declare_dram_parameter("w2", [DFF, D], BF16, isOutput=False)
    bq_ext = nc.declare_dram_parameter("bq", [D], F32, isOutput=False)
    bk_ext = nc.declare_dram_parameter("bk", [D], F32, isOutput=False)
    bv_ext = nc.declare_dram_parameter("bv", [D], F32, isOutput=False)
    bo_ext = nc.declare_dram_parameter("bo", [D], F32, isOutput=False)
    b1_ext = nc.declare_dram_parameter("b1", [DFF], F32, isOutput=False)
    b2_ext = nc.declare_dram_parameter("b2", [D], F32, isOutput=False)
    bias0_ext = nc.declare_dram_parameter("bias0", [128, 512], BF16, isOutput=False)
    biasr_ext = nc.declare_dram_parameter("biasr", [128, 512], BF16, isOutput=False)
    out_ext = nc.declare_dram_parameter("out", [SEG, D], F32, isOutput=True)

    with tile.TileContext(nc) as tc:
        _body(nc, tc, locals())
    if split:
        _split_multi_sync(nc)
    return nc


def _ln_stats(nc, ln, x_ap, eps_tile):
    """bn_stats + rstd via exp(-0.5*ln(var+eps)) (stays in the Exp act table).
    Returns (nmr, rstd) tiles: h = x*rstd + nmr."""
    stats = ln.tile([128, 2, 6], F32, tag="ln_stats")
    xr = x_ap.rearrange("p (s f) -> p s f", f=512)
    for s in range(2):
        nc.vector.bn_stats(out=stats[:, s, :], in_=xr[:, s, :])
    mv = ln.tile([128, 2], F32, tag="ln_mv")
    nc.vector.bn_aggr(out=mv[:, :], in_=stats[:, :, :])
    lnv = ln.tile([128, 1], F32, tag="ln_lnv")
    nc.scalar.activation(out=lnv, in_=mv[:, 1:2], func=AF.Ln, bias=eps_tile, scale=1.0)
    rstd = ln.tile([128, 1], F32, tag="ln_rstd")
    nc.scalar.activation(out=rstd, in_=lnv, func=AF.Exp, bias=0.0, scale=-0.5)
    nmr = ln.tile([128, 1], F32, tag="ln_nmr")
    # nmr = -mean * rstd in one DVE pass
    nc.vector.tensor_scalar(nmr, mv[:, 0:1], rstd, -1.0,
                            mybir.AluOpType.mult, mybir.AluOpType.mult)
    return nmr, rstd


def _body(nc, tc, ext):
    st = tc.tile_pool  # shorthand

    with (
        st(name="const", bufs=1) as const,
        st(name="resid", bufs=1) as resid,
        st(name="ln", bufs=3) as ln,
        st(name="pmm", bufs=2, space="PSUM") as pmm,
        st(name="pscore", bufs=3, space="PSUM") as pscore,
        st(name="ptr", bufs=2, space="PSUM") as ptr,
        st(name="pctx", bufs=1, space="PSUM") as pctx,
    ):
        def pmac(out, a_fn, b_fn, nk=8):
            """Accumulating K-chunk matmul: paired chunks under DoubleRow,
            per-chunk otherwise. a_fn/b_fn map a chunk slice to the operand AP."""
            if DR is not None:
                for c in range(0, nk, 2):
                    nc.tensor.matmul(out, a_fn(slice(c, c + 2)), b_fn(slice(c, c + 2)),
                                     start=(c == 0), stop=(c == nk - 2), perf_mode=DR)
            else:
                for c in range(nk):
                    nc.tensor.matmul(out, a_fn(c), b_fn(c),
                                     start=(c == 0), stop=(c == nk - 1))

        def ptile(pool, shape, tg):
            return pool.tile(shape, F32, tag=tg, name="pst_" + tg)

        def ptile_bf(pool, shape, tg):
            return pool.tile(shape, BF16, tag=tg, name="pstb_" + tg)

        # ---- constants ----
        ident = const.tile([128, 128], BF16)
        make_identity(nc, ident)
        x_sb = const.tile([128, NT, D], F32)
        xr = ext["x_ext"].rearrange("(t p) d -> p t d", p=128)
        for t in range(NT):
            nc.sync.dma_start(out=x_sb[:, t, :], in_=xr[:, t, :])
        eps_tile = const.tile([128, 1], F32)
        nc.vector.memset(eps_tile, LN_EPS)
        bq_sb = const.tile([128, 8], F32)
        nc.sync.dma_start(out=bq_sb, in_=ext["bq_ext"].rearrange("(j p) -> p j", p=128))
        bk_sb = const.tile([128, 8], F32)
        nc.sync.dma_start(out=bk_sb, in_=ext["bk_ext"].rearrange("(j p) -> p j", p=128))
        b1_sb = const.tile([128, 32], F32)
        nc.sync.dma_start(out=b1_sb, in_=ext["b1_ext"].rearrange("(j p) -> p j", p=128))

        def bcast(name):
            t_ = const.tile([128, D], F32, tag=f"bc_{name}")
            src = ext[f"{name}_ext"][:]
            ap = bass.AP(tensor=src.tensor, offset=src.offset,
                         ap=[[0, 128]] + list(src.ap))
            nc.sync.dma_start(out=t_, in_=ap)
            return t_

        bv_bc = bcast("bv")
        bo_bc = bcast("bo")
        b2_bc = bcast("b2")
        bias0 = const.tile([128, 512], BF16)
        nc.sync.dma_start(out=bias0, in_=ext["bias0_ext"][:, :])
        biasr = const.tile([128, 512], BF16)
        nc.sync.dma_start(out=biasr, in_=ext["biasr_ext"][:, :])

        x2_sb = resid.tile([128, 4, D], F32)
        h2T = resid.tile([128, 8, SEG], BF16)

        # ---- PE warmup: prime the p-state ramp while x loads (PE must stay
        # busy ~3us continuously to reach full clock, and idle gaps re-throttle
        # it to half clock) ----
        for i in range(24):
            wu = ptile_bf(ptr, [128, 512], "ptr")
            for q4 in range(4):
                nc.tensor.transpose(wu[:, q4 * 128:(q4 + 1) * 128], ident, ident)

        with st(name="attnw", bufs=1) as attnw, st(name="scr", bufs=3) as scr, \
             st(name="soft", bufs=5) as soft:
            wo_sb = attnw.tile([128, 8, D], FP8)
            qT = attnw.tile([128, 8, SEG], BF16)
            kT = attnw.tile([128, 8, T], BF16)
            v_sb = attnw.tile([128, NT, D], BF16)
            ctxT = attnw.tile([128, 8, SEG], FP8)

            with st(name="qkvw", bufs=1) as qkvw:
                wq_sb = qkvw.tile([128, 8, D], FP8)
                nc.gpsimd.dma_start(out=wq_sb, in_=ext["wq_ext"].rearrange("(k p) n -> p k n", p=128))
                wk_sb = qkvw.tile([128, 8, D], FP8)
                nc.gpsimd.dma_start(out=wk_sb, in_=ext["wk_ext"].rearrange("(k p) n -> p k n", p=128))
                wv_sb = qkvw.tile([128, 8, D], FP8)
                nc.gpsimd.dma_start(out=wv_sb, in_=ext["wv_ext"].rearrange("(k p) n -> p k n", p=128))
                nc.gpsimd.dma_start(out=wo_sb, in_=ext["wo_ext"].rearrange("(k p) n -> p k n", p=128))
                hT = qkvw.tile([128, 8, T], FP8)

                # ---- LN1: stats stream in as x tiles land (DVE only, no
                # cross-engine ping-pong), then ONE batched rstd for all 5
                # tiles, then apply/transpose/V per tile.
                mv_all = qkvw.tile([128, NT, 2], F32)
                for t in range(NT):
                    stats = ln.tile([128, 2, 6], F32, tag="ln_stats")
                    xr = x_sb[:, t, :].rearrange("p (s f) -> p s f", f=512)
                    for s in range(2):
                        nc.vector.bn_stats(out=stats[:, s, :], in_=xr[:, s, :])
                    nc.vector.bn_aggr(out=mv_all[:, t, :], in_=stats[:, :, :])
                lnv5 = qkvw.tile([128, NT], F32)
                nc.scalar.activation(out=lnv5, in_=mv_all[:, :, 1], func=AF.Ln,
                                     bias=eps_tile, scale=1.0)
                rstd5 = qkvw.tile([128, NT], F32)
                nc.scalar.activation(out=rstd5, in_=lnv5, func=AF.Exp, bias=0.0, scale=-0.5)
                nmr5 = qkvw.tile([128, NT], F32)
                nc.vector.scalar_tensor_tensor(out=nmr5, in0=mv_all[:, :, 0], scalar=-1.0,
                                               in1=rstd5, op0=mybir.AluOpType.mult,
                                               op1=mybir.AluOpType.mult)
                for t in range(NT):
                    h_t = scr.tile([128, D], BF16, tag="h_t")
                    nc.scalar.activation(out=h_t, in_=x_sb[:, t, :], func=AF.Identity,
                                         bias=nmr5[:, t:t + 1], scale=rstd5[:, t:t + 1])
                    for g in range(2):
                        pt = ptile_bf(ptr, [128, 512], "ptr")
                        for jj in range(4):
                            j = g * 4 + jj
                            nc.tensor.transpose(pt[:, jj * 128:(jj + 1) * 128],
                                                h_t[:, j * 128:(j + 1) * 128], ident)
                        dst = hT[:, g * 4:(g + 1) * 4, t * 128:(t + 1) * 128]
                        if (t * 2 + g) % 2 == 0:
                            nc.vector.tensor_copy(out=dst, in_=pt.rearrange("p (j c) -> p j c", j=4))
                        else:
                            nc.scalar.copy(out=dst, in_=pt.rearrange("p (j c) -> p j c", j=4))
                    # V for this tile: lhsT = hT chunk (pairs under DoubleRow)
                    for n in range(2):
                        pv = ptile(pmm, [128, 512], "mm")
                        pmac(pv, lambda c, t=t: hT[:, c, t * 128:(t + 1) * 128],
                             lambda c, n=n: wv_sb[:, c, n * 512:(n + 1) * 512])
                        nc.vector.tensor_add(v_sb[:, t, n * 512:(n + 1) * 512], pv,
                                             bv_bc[:, n * 512:(n + 1) * 512])

                # residual bias pre-add (off critical path; ordered after LN1 reads)
                for t in range(4):
                    nc.vector.tensor_add(x_sb[:, t + 1, :], x_sb[:, t + 1, :], bo_bc)

                # ---- Q/K projections (DoubleRow fp8) ----
                for j in range(8):
                    pq = ptile(pmm, [128, SEG], "mm")
                    pmac(pq, lambda c, j=j: wq_sb[:, c, j * 128:(j + 1) * 128],
                         lambda c: hT[:, c, HALO:T])
                    nc.scalar.activation(out=qT[:, j, :], in_=pq, func=AF.Identity,
                                         bias=bq_sb[:, j:j + 1], scale=1.0)
                for j in range(8):
                    pk = ptile(pmm, [128, SEG], "mm")
                    pmac(pk, lambda c, j=j: wk_sb[:, c, j * 128:(j + 1) * 128],
                         lambda c: hT[:, c, HALO:T])
                    nc.scalar.activation(out=kT[:, j, HALO:T], in_=pk, func=AF.Identity,
                                         bias=bk_sb[:, j:j + 1], scale=1.0)
                    pkh = ptile(pctx, [128, HALO], "pctx")
                    pmac(pkh, lambda c, j=j: wk_sb[:, c, j * 128:(j + 1) * 128],
                         lambda c: hT[:, c, 0:HALO])
                    nc.vector.tensor_scalar_add(kT[:, j, 0:HALO], pkh, bk_sb[:, j:j + 1])

            # ---- attention, software-pipelined: scores run LAG ahead of
            # transpose+ctx so the PE stream never waits on the softmax chain.
            LAG = 2

            def s_block(qb, j2):
                """PE: mask bias matmul + 2 head score matmuls into one PSUM
                tile; ACT: exp; DVE: rowsum, reciprocal, normalize (bf16)."""
                bias_t = bias0 if qb == 0 else biasr
                ps = ptile(pscore, [128, 512], "psc")
                for hi, r in enumerate((0, 64)):
                    sl = slice(hi * 256, (hi + 1) * 256)
                    nc.tensor.matmul(ps[:, sl], ident, bias_t[:, sl],
                                     start=True, stop=False)
                    nc.tensor.matmul(ps[:, sl],
                                     qT[r:r + 64, j2, qb * 128:(qb + 1) * 128],
                                     kT[r:r + 64, j2, qb * 128:qb * 128 + 256],
                                     start=False, stop=True)
                p_pair = soft.tile([128, 512], BF16, tag="p_pair")
                nc.scalar.activation(out=p_pair, in_=ps, func=AF.Exp,
                                     bias=0.0, scale=1.0)
                rs = soft.tile([128, 2], F32, tag="rs")
                nc.vector.reduce_sum(out=rs, in_=p_pair.rearrange("p (h k) -> p h k", h=2),
                                     axis=AX.X)
                rinv = soft.tile([128, 2], F32, tag="rinv")
                nc.vector.reciprocal(rinv, rs)
                for hi in range(2):
                    nc.vector.tensor_scalar_mul(p_pair[:, hi * 256:(hi + 1) * 256],
                                                p_pair[:, hi * 256:(hi + 1) * 256],
                                                rinv[:, hi:hi + 1])
                return p_pair

            def tc_block(qb, j2, p_pair):
                """PE: transpose probs, ctx matmuls; Pool: pT evict;
                ACT: ctxT evict (fp8)."""
                ptp = ptile_bf(ptr, [128, 512], "ptr")
                for q4 in range(4):
                    nc.tensor.transpose(ptp[:, q4 * 128:(q4 + 1) * 128],
                                        p_pair[:, q4 * 128:(q4 + 1) * 128], ident)
                pT = soft.tile([128, 512], BF16, tag="pT")
                if j2 % 2 == 0:
                    nc.vector.tensor_copy(out=pT, in_=ptp)
                else:
                    nc.scalar.copy(out=pT, in_=ptp)
                pc = ptile(pctx, [128, 128], "pctx")
                for hi, r in enumerate((0, 64)):
                    h = 2 * j2 + hi
                    for half in range(2):
                        kb = qb + half
                        nc.tensor.matmul(pc[r:r + 64, :],
                                         v_sb[:, kb, h * 64:(h + 1) * 64],
                                         pT[:, (hi * 2 + half) * 128:(hi * 2 + half + 1) * 128],
                                         start=(half == 0), stop=(half == 1),
                                         tile_position=(0, r))
                nc.scalar.copy(out=ctxT[:, j2, qb * 128:(qb + 1) * 128], in_=pc)

            def oproj_ln2(t):
                # out-projection (DoubleRow fp8) + residual for this token block
                for n in range(2):
                    po = ptile(pmm, [128, 512], "mm")
                    pmac(po, lambda c, t=t: ctxT[:, c, t * 128:(t + 1) * 128],
                         lambda c, n=n: wo_sb[:, c, n * 512:(n + 1) * 512])
                    sl = slice(n * 512, (n + 1) * 512)
                    nc.vector.tensor_add(x2_sb[:, t, sl], po, x_sb[:, t + 1, sl])

                # LN2 + transpose for this token block
                nmr2, rstd2 = _ln_stats(nc, ln, x2_sb[:, t, :], eps_tile)
                h2_t = scr.tile([128, D], BF16, tag="h2_t")
                nc.scalar.activation(out=h2_t, in_=x2_sb[:, t, :], func=AF.Identity,
                                     bias=nmr2, scale=rstd2)
                for g in range(2):
                    pt = ptile_bf(ptr, [128, 512], "ptr")
                    for jj in range(4):
                        j = g * 4 + jj
                        nc.tensor.transpose(pt[:, jj * 128:(jj + 1) * 128],
                                            h2_t[:, j * 128:(j + 1) * 128], ident)
                    dst = h2T[:, g * 4:(g + 1) * 4, t * 128:(t + 1) * 128]
                    if (t * 2 + g) % 2 == 0:
                        nc.vector.tensor_copy(out=dst, in_=pt.rearrange("p (j c) -> p j c", j=4))
                    else:
                        nc.scalar.copy(out=dst, in_=pt.rearrange("p (j c) -> p j c", j=4))
                # final-residual bias pre-add (after LN2 consumed x2[t])
                nc.vector.tensor_add(x2_sb[:, t, :], x2_sb[:, t, :], b2_bc)

            pending = []
            for it in range(32 + LAG):
                if it < 32:
                    qb, j2 = divmod(it, 8)
                    pending.append((qb, j2, s_block(qb, j2)))
                if it >= LAG:
                    qb2, j22, pp = pending.pop(0)
                    tc_block(qb2, j22, pp)
                    if j22 == 7:
                        oproj_ln2(qb2)

        # ---- FFN (bf16) ----
        with st(name="ffnw", bufs=1) as ffnw, st(name="w1p", bufs=2) as w1p, \
             st(name="outp", bufs=2) as outp:
            gT = ffnw.tile([128, 32, SEG], BF16)
            w2_sb = ffnw.tile([128, 32, D], BF16)

            w2r = ext["w2_ext"].rearrange("(c p) n -> p c n", p=128)
            for c in range(4):
                nc.gpsimd.dma_start(out=w2_sb[:, c * 8:(c + 1) * 8, :],
                                    in_=w2r[:, c * 8:(c + 1) * 8, :])

            w1r = ext["w1_ext"].rearrange("(k p) n -> p k n", p=128)
            for c in range(4):
                w1c = w1p.tile([128, 8, 1024], BF16, tag="w1c")
                nc.gpsimd.dma_start(out=w1c, in_=w1r[:, :, c * 1024:(c + 1) * 1024])
                for jj in range(8):
                    jdff = c * 8 + jj
                    pg = ptile(pmm, [128, SEG], "mm")
                    for k in range(8):
                        nc.tensor.matmul(pg, w1c[:, k, jj * 128:(jj + 1) * 128],
                                         h2T[:, k, :], start=(k == 0), stop=(k == 7))
                    nc.scalar.activation(out=gT[:, jdff, :], in_=pg, func=AF.Gelu_apprx_tanh,
                                         bias=b1_sb[:, jdff:jdff + 1], scale=1.0)

            outr = ext["out_ext"].rearrange("(t p) d -> p t d", p=128)
            for t in range(4):
                o_t = outp.tile([128, D], F32, tag="o_t")
                for n in range(2):
                    py = ptile(pmm, [128, 512], "mm")
                    for k in range(32):
                        nc.tensor.matmul(py, gT[:, k, t * 128:(t + 1) * 128],
                                         w2_sb[:, k, n * 512:(n + 1) * 512],
                                         start=(k == 0), stop=(k == 31))
                    sl = slice(n * 512, (n + 1) * 512)
                    nc.vector.tensor_add(o_t[:, sl], py, x2_sb[:, t, sl])
                nc.sync.dma_start(out=outr[:, t, :], in_=o_t)


def _host_prep(x, Wq, bq, Wk, bk, Wv, bv, Wo, bo, W1, b1, W2, b2,
               ln1_w, ln1_b, ln2_w, ln2_b):
    bf = ml_dtypes.bfloat16
    f8 = ml_dtypes.float8_e4m3fn if _USE_FP8 else bf

    def q8(a):
        return np.ascontiguousarray(
            np.clip(np.asarray(a, np.float32), -240.0, 240.0).astype(f8))

    sc = 1.0 / np.sqrt(DH)
    wq_eff = q8((ln1_w[:, None] * Wq) * sc)
    bq_eff = ((bq + ln1_b @ Wq) * sc).astype(np.float32)
    wk_eff = q8(ln1_w[:, None] * Wk)
    bk_eff = (bk + ln1_b @ Wk).astype(np.float32)
    wv_eff = q8(ln1_w[:, None] * Wv)
    bv_eff = (bv + ln1_b @ Wv).astype(np.float32)
    w1_eff = (ln2_w[:, None] * W1).astype(bf)
    b1_eff = (b1 + ln2_b @ W1).astype(np.float32)

    r = np.arange(128)[:, None]
    c = np.arange(128)[None, :]
    left = np.where(c >= r, 0.0, NEG).astype(np.float32)
    diag = np.where(c <= r, 0.0, NEG).astype(np.float32)
    biasr = np.concatenate([left, diag, left, diag], axis=1).astype(bf)
    bias0_halo = np.concatenate(
        [np.full((128, 128), NEG, np.float32), diag,
         np.full((128, 128), NEG, np.float32), diag], axis=1).astype(bf)

    shared = {
        "wq": wq_eff, "wk": wk_eff, "wv": wv_eff,
        "wo": q8(Wo),
        "w1": w1_eff, "w2": np.ascontiguousarray(W2.astype(bf)),
        "bq": bq_eff, "bk": bk_eff, "bv": bv_eff,
        "bo": bo.astype(np.float32), "b1": b1_eff, "b2": b2.astype(np.float32),
        "biasr": biasr,
    }
    in_maps = []
    for core in range(NSEG):
        b_, s_ = core // 4, core % 4
        if s_ == 0:
            seg = np.concatenate(
                [np.zeros((HALO, D), np.float32), x[b_, 0:SEG]], axis=0)
            bias0 = bias0_halo
        else:
            seg = x[b_, s_ * SEG - HALO: (s_ + 1) * SEG]
            bias0 = biasr
        m = dict(shared)
        m["x"] = np.ascontiguousarray(seg.astype(np.float32))
        m["bias0"] = bias0
        in_maps.append(m)
    return in_maps


def kernel(**inputs):
    from concourse.bass_utils import run_bass_kernel_spmd

    if "nc" not in _CACHED:
        _CACHED["nc"] = _build()
    nc = _CACHED["nc"]

    in_maps = _host_prep(**{k: np.asarray(v) for k, v in inputs.items()})
    trace = bool(int(os.environ.get("KERNEL_TRACE", "0")))
    res = run_bass_kernel_spmd(nc, in_maps, list(range(NSEG)), trace=trace)
    kernel.last_results = res

    x = np.asarray(inputs["x"])
    out = np.empty((B, L, D), np.float32)
    for core in range(NSEG):
        b_, s_ = core // 4, core % 4
        out[b_, s_ * SEG:(s_ + 1) * SEG] = res.results[core]["out"]
    return out


# revision 23
# speedup vs baseline: 1.0671x; 1.0671x over previous
"""Causal local-window (W=128) attention block + FFN, distributed over 8 TRN2
NeuronCores with ZERO collectives.

Sharding: (B=2, L=2048) tokens are split into 8 contiguous segments of 512
tokens (4 per batch element). Each core receives its 512 owned tokens plus a
128-token left halo (zero-padded for the first segment of each batch) and
recomputes the halo's K/V locally — the sliding window (j in [i-128, i]) never
crosses more than 128 tokens back, so no cross-core communication is needed.

Key implementation choices (v2):
  - fp8(e4m3) weights + activations with MatmulPerfMode.DoubleRow for the
    QKV projections and the attention out-projection (2 K-chunks contracted
    per pass). FFN + attention internals stay bf16 (fp8 there would break
    the 2e-2 error budget; measured headroom: attn-side fp8 = 1.4e-2).
  - The additive sliding-window mask is applied on the PE: an identity
    matmul writes the mask into PSUM (start=True) and the score matmuls
    accumulate on top — no f32 DVE bias-add in the softmax path.
  - exp() runs on the Scalar engine straight from PSUM; row sums + the
    1/sum normalization run on DVE in bf16 (2x mode); the transposed-
    probability PSUM eviction runs on GpSimd; ctx eviction on Scalar.
    The attention inner loop is software-pipelined (scores run 2
    iterations ahead of transpose+ctx) so the PE never idles — TRN2 drops
    the PE clock 2x for ~3us after any idle gap.
  - LayerNorm rstd = exp(-0.5*ln(var+eps)): Ln, Exp, Identity and Copy all
    live in one activation table, so the Scalar engine never reloads
    tables inside the attention phase (a reload is 1.3us).
"""

import os
import numpy as np
import ml_dtypes

import concourse.bass as bass
import concourse.mybir as mybir
import concourse.tile as tile
from concourse.masks import make_identity
from bass_rust import ScopedClock

# ---------------------------------------------------------------------------
# Workarounds for the walrus build in this container, which accepts at most
# ONE sync-wait and ONE sync-update per instruction. Tile attaches one wait
# per out-of-date producer clock and one update per consumer engine, so any
# nontrivial Tile kernel violates this. Fix by splitting the extras onto
# standalone InstEventSemaphore instructions on the same engine: waits go
# immediately BEFORE the instruction, updates immediately AFTER (each engine
# executes its stream in order, so semantics are preserved).
_split_counter = [0]


def _split_multi_sync(nc):
    for f in nc.m.functions:
        for bb in f.blocks:
            il = list(bb.instructions)
            new = []
            changed = False
            for inst in il:
                si = inst.sync_info
                waits = list(si.on_wait) if si and si.on_wait else []
                upds = list(si.on_update) if si and si.on_update else []
                if len(waits) > 1:
                    changed = True
                    for w in waits[:-1]:
                        _split_counter[0] += 1
                        new.append(mybir.InstEventSemaphore(
                            name=f"I-wsplit-{_split_counter[0]}",
                            engine=inst.engine, ins=[], outs=[],
                            sync_info=mybir.SyncInfo(on_wait=[w], on_update=[]),
                        ))
                    si.on_wait = [waits[-1]]
                new.append(inst)
                if len(upds) > 1:
                    changed = True
                    si.on_update = [upds[0]]
                    for u in upds[1:]:
                        _split_counter[0] += 1
                        new.append(mybir.InstEventSemaphore(
                            name=f"I-usplit-{_split_counter[0]}",
                            engine=inst.engine, ins=[], outs=[],
                            sync_info=mybir.SyncInfo(on_wait=[], on_update=[u]),
                        ))
            if changed:
                bb.instructions = new


def _patched_drain_and_barrier(self, tick_clock, wait_clock):
    # Tile's kernel-tail drain carries one wait per logical processor; split
    # them into standalone single-wait SP instructions instead.
    nc = self.nc
    drain_inst = nc.sync.drain()
    wait_clock.add_sem_waits(drain_inst.ins, ScopedClock({None: tick_clock.global_clock}))
    si = drain_inst.ins.sync_info
    waits = list(si.on_wait or [])
    if len(waits) > 1:
        si.on_wait = []
        handles = {}
        for s in self.sems.allocated().values():
            nm = getattr(s, 'ant_name', None) or getattr(s, 'name', None)
            handles[nm] = s
        for w in waits:
            assert w.wait_mode == 'sem-ge-imm', w
            nc.sync.wait_ge(handles[w.ant_name], w.wait_value)
    nc.all_engine_barrier()
    assert self.sems is not None
    popped = nc._tile_sem_poison_stack.pop()
    assert popped is self._sem_poison
    nc.clear_and_free_semaphores(list(self.sems.allocated().values()))
    nc.all_engine_barrier()


tile.TileContext._drain_and_barrier = _patched_drain_and_barrier

F32 = mybir.dt.float32
BF16 = mybir.dt.bfloat16
AF = mybir.ActivationFunctionType
AX = mybir.AxisListType

# debug toggles (read at build time)
_USE_FP8 = os.environ.get("K_FP8", "0") == "1"
_USE_DR = os.environ.get("K_DR", "1") == "1" and _USE_FP8
FP8 = mybir.dt.float8e4 if _USE_FP8 else mybir.dt.bfloat16
DR = mybir.MatmulPerfMode.DoubleRow if _USE_DR else None

B, L, D = 2, 2048, 1024
NH, DH = 16, 64
DFF = 4096
WIN = 128
SEG = 512          # owned tokens per core
HALO = 128
T = SEG + HALO     # 640 local tokens
NT = T // 128      # 5 local token tiles
NSEG = 8           # cores
NEG = -1.0e30
LN_EPS = 1e-5

_CACHED = {}


def _build(split=True):
    nc = bass.Bass()
    x_ext = nc.declare_dram_parameter("x", [T, D], F32, isOutput=False)
    wq_ext = nc.declare_dram_parameter("wq", [D, D], FP8, isOutput=False)
    wk_ext = nc.declare_dram_parameter("wk", [D, D], FP8, isOutput=False)
    wv_ext = nc.declare_dram_parameter("wv", [D, D], FP8, isOutput=False)
    wo_ext = nc.declare_dram_parameter("wo", [D, D], FP8, isOutput=False)
    w1_ext = nc.declare_dram_parameter("w1", [D, DFF], BF16, isOutput=False)
    w2_ext = nc.declare_dram_parameter("w2", [DFF, D], BF16, isOutput=False)
    bq_ext = nc.declare_dram_parameter("bq", [D], F32, isOutput=False)
    bk_ext = nc.declare_dram_parameter("bk", [D], F32, isOutput=False)
    bv_ext = nc.declare_dram_parameter("bv", [D], F32, isOutput=False)
    bo_ext = nc.declare_dram_parameter("bo", [D], F32, isOutput=False)
    b1_ext = nc.declare_dram_parameter("b1", [DFF], F32, isOutput=False)
    b2_ext = nc.declare_dram_parameter("b2", [D], F32, isOutput=False)
    bias0_ext = nc.declare_dram_parameter("bias0", [128, 512], BF16, isOutput=False)
    biasr_ext = nc.declare_dram_parameter("biasr", [128, 512], BF16, isOutput=False)
    out_ext = nc.declare_dram_parameter("out", [SEG, D], F32, isOutput=True)

    with tile.TileContext(nc) as tc:
        _body(nc, tc, locals())
    if split:
        _split_multi_sync(nc)
    return nc


def _ln_stats(nc, ln, x_ap, eps_tile):
    """bn_stats + rstd via exp(-0.5*ln(var+eps)) (stays in the Exp act table).
    Returns (nmr, rstd) tiles: h = x*rstd + nmr."""
    stats = ln.tile([128, 2, 6], F32, tag="ln_stats")
    xr = x_ap.rearrange("p (s f) -> p s f", f=512)
    for s in range(2):
        nc.vector.bn_stats(out=stats[:, s, :], in_=xr[:, s, :])
    mv = ln.tile([128, 2], F32, tag="ln_mv")
    nc.vector.bn_aggr(out=mv[:, :], in_=stats[:, :, :])
    lnv = ln.tile([128, 1], F32, tag="ln_lnv")
    nc.scalar.activation(out=lnv, in_=mv[:, 1:2], func=AF.Ln, bias=eps_tile, scale=1.0)
    rstd = ln.tile([128, 1], F32, tag="ln_rstd")
    nc.scalar.activation(out=rstd, in_=lnv, func=AF.Exp, bias=0.0, scale=-0.5)
    nmr = ln.tile([128, 1], F32, tag="ln_nmr")
    # nmr = -mean * rstd in one DVE pass
    nc.vector.tensor_scalar(nmr, mv[:, 0:1], rstd, -1.0,
                            mybir.AluOpType.mult, mybir.AluOpType.mult)
    return nmr, rstd


def _body(nc, tc, ext):
    st = tc.tile_pool  # shorthand

    with (
        st(name="const", bufs=1) as const,
        st(name="resid", bufs=1) as resid,
        st(name="ln", bufs=3) as ln,
        st(name="pmm", bufs=2, space="PSUM") as pmm,
        st(name="pscore", bufs=3, space="PSUM") as pscore,
        st(name="ptr", bufs=2, space="PSUM") as ptr,
        st(name="pctx", bufs=1, space="PSUM") as pctx,
    ):
        def pmac(out, a_fn, b_fn, nk=8):
            """Accumulating K-chunk matmul: paired chunks under DoubleRow,
            per-chunk otherwise. a_fn/b_fn map a chunk slice to the operand AP."""
            if DR is not None:
                for c in range(0, nk, 2):
                    nc.tensor.matmul(out, a_fn(slice(c, c + 2)), b_fn(slice(c, c + 2)),
                                     start=(c == 0), stop=(c == nk - 2), perf_mode=DR)
            else:
                for c in range(nk):
                    nc.tensor.matmul(out, a_fn(c), b_fn(c),
                                     start=(c == 0), stop=(c == nk - 1))

        def ptile(pool, shape, tg):
            return pool.tile(shape, F32, tag=tg, name="pst_" + tg)

        def ptile_bf(pool, shape, tg):
            return pool.tile(shape, BF16, tag=tg, name="pstb_" + tg)

        # ---- constants ----
        ident = const.tile([128, 128], BF16)
        make_identity(nc, ident)
        x_sb = const.tile([128, NT, D], F32)
        xr = ext["x_ext"].rearrange("(t p) d -> p t d", p=128)
        for t in range(NT):
            nc.sync.dma_start(out=x_sb[:, t, :], in_=xr[:, t, :])
        eps_tile = const.tile([128, 1], F32)
        nc.vector.memset(eps_tile, LN_EPS)
        bq_sb = const.tile([128, 8], F32)
        nc.sync.dma_start(out=bq_sb, in_=ext["bq_ext"].rearrange("(j p) -> p j", p=128))
        bk_sb = const.tile([128, 8], F32)
        nc.sync.dma_start(out=bk_sb, in_=ext["bk_ext"].rearrange("(j p) -> p j", p=128))
        b1_sb = const.tile([128, 32], F32)
        nc.sync.dma_start(out=b1_sb, in_=ext["b1_ext"].rearrange("(j p) -> p j", p=128))

        def bcast(name):
            t_ = const.tile([128, D], F32, tag=f"bc_{name}")
            src = ext[f"{name}_ext"][:]
            ap = bass.AP(tensor=src.tensor, offset=src.offset,
                         ap=[[0, 128]] + list(src.ap))
            nc.sync.dma_start(out=t_, in_=ap)
            return t_

        bv_bc = bcast("bv")
        bo_bc = bcast("bo")
        b2_bc = bcast("b2")
        bias0 = const.tile([128, 512], BF16)
        nc.sync.dma_start(out=bias0, in_=ext["bias0_ext"][:, :])
        biasr = const.tile([128, 512], BF16)
        nc.sync.dma_start(out=biasr, in_=ext["biasr_ext"][:, :])

        x2_sb = resid.tile([128, 4, D], F32)
        h2T = resid.tile([128, 8, SEG], BF16)

        # ---- PE warmup: prime the p-state ramp while x loads (PE must stay
        # busy ~3us continuously to reach full clock, and idle gaps re-throttle
        # it to half clock) ----
        for i in range(24):
            wu = ptile_bf(ptr, [128, 512], "ptr")
            for q4 in range(4):
                nc.tensor.transpose(wu[:, q4 * 128:(q4 + 1) * 128], ident, ident)

        with st(name="attnw", bufs=1) as attnw, st(name="scr", bufs=3) as scr, \
             st(name="soft", bufs=5) as soft:
            wo_sb = attnw.tile([128, 8, D], FP8)
            qT = attnw.tile([128, 8, SEG], BF16)
            kT = attnw.tile([128, 8, T], BF16)
            v_sb = attnw.tile([128, NT, D], BF16)
            ctxT = attnw.tile([128, 8, SEG], FP8)

            with st(name="qkvw", bufs=1) as qkvw:
                wq_sb = qkvw.tile([128, 8, D], FP8)
                nc.gpsimd.dma_start(out=wq_sb, in_=ext["wq_ext"].rearrange("(k p) n -> p k n", p=128))
                wk_sb = qkvw.tile([128, 8, D], FP8)
                nc.gpsimd.dma_start(out=wk_sb, in_=ext["wk_ext"].rearrange("(k p) n -> p k n", p=128))
                wv_sb = qkvw.tile([128, 8, D], FP8)
                nc.gpsimd.dma_start(out=wv_sb, in_=ext["wv_ext"].rearrange("(k p) n -> p k n", p=128))
                nc.gpsimd.dma_start(out=wo_sb, in_=ext["wo_ext"].rearrange("(k p) n -> p k n", p=128))
                hT = qkvw.tile([128, 8, T], FP8)

                # ---- LN1: stats stream in as x tiles land (DVE only, no
                # cross-engine ping-pong), then ONE batched rstd for all 5
                # tiles, then apply/transpose/V per tile.
                mv_all = qkvw.tile([128, NT, 2], F32)
                for t in range(NT):
                    stats = ln.tile([128, 2, 6], F32, tag="ln_stats")
                    xr = x_sb[:, t, :].rearrange("p (s f) -> p s f", f=512)
                    for s in range(2):
                        nc.vector.bn_stats(out=stats[:, s, :], in_=xr[:, s, :])
                    nc.vector.bn_aggr(out=mv_all[:, t, :], in_=stats[:, :, :])
                lnv5 = qkvw.tile([128, NT], F32)
                nc.scalar.activation(out=lnv5, in_=mv_all[:, :, 1], func=AF.Ln,
                                     bias=eps_tile, scale=1.0)
                rstd5 = qkvw.tile([128, NT], F32)
                nc.scalar.activation(out=rstd5, in_=lnv5, func=AF.Exp, bias=0.0, scale=-0.5)
                nmr5 = qkvw.tile([128, NT], F32)
                nc.vector.scalar_tensor_tensor(out=nmr5, in0=mv_all[:, :, 0], scalar=-1.0,
                                               in1=rstd5, op0=mybir.AluOpType.mult,
                                               op1=mybir.AluOpType.mult)
                for t in range(NT):
                    h_t = scr.tile([128, D], BF16, tag="h_t")
                    nc.scalar.activation(out=h_t, in_=x_sb[:, t, :], func=AF.Identity,
                                         bias=nmr5[:, t:t + 1], scale=rstd5[:, t:t + 1])
                    for g in range(2):
                        pt = ptile_bf(ptr, [128, 512], "ptr")
                        for jj in range(4):
                            j = g * 4 + jj
                            nc.tensor.transpose(pt[:, jj * 128:(jj + 1) * 128],
                                                h_t[:, j * 128:(j + 1) * 128], ident)
                        dst = hT[:, g * 4:(g + 1) * 4, t * 128:(t + 1) * 128]
                        if (t * 2 + g) % 2 == 0:
                            nc.vector.tensor_copy(out=dst, in_=pt.rearrange("p (j c) -> p j c", j=4))
                        else:
                            nc.scalar.copy(out=dst, in_=pt.rearrange("p (j c) -> p j c", j=4))
                    # V for this tile: lhsT = hT chunk (pairs under DoubleRow)
                    for n in range(2):
                        pv = ptile(pmm, [128, 512], "mm")
                        pmac(pv, lambda c, t=t: hT[:, c, t * 128:(t + 1) * 128],
                             lambda c, n=n: wv_sb[:, c, n * 512:(n + 1) * 512])
                        nc.vector.tensor_add(v_sb[:, t, n * 512:(n + 1) * 512], pv,
                                             bv_bc[:, n * 512:(n + 1) * 512])

                # residual bias pre-add (off critical path; ordered after LN1 reads)
                for t in range(4):
                    nc.vector.tensor_add(x_sb[:, t + 1, :], x_sb[:, t + 1, :], bo_bc)

                # ---- Q/K projections (DoubleRow fp8) ----
                for j in range(8):
                    pq = ptile(pmm, [128, SEG], "mm")
                    pmac(pq, lambda c, j=j: wq_sb[:, c, j * 128:(j + 1) * 128],
                         lambda c: hT[:, c, HALO:T])
                    nc.scalar.activation(out=qT[:, j, :], in_=pq, func=AF.Identity,
                                         bias=bq_sb[:, j:j + 1], scale=1.0)
                for j in range(8):
                    pk = ptile(pmm, [128, SEG], "mm")
                    pmac(pk, lambda c, j=j: wk_sb[:, c, j * 128:(j + 1) * 128],
                         lambda c: hT[:, c, HALO:T])
                    nc.scalar.activation(out=kT[:, j, HALO:T], in_=pk, func=AF.Identity,
                                         bias=bk_sb[:, j:j + 1], scale=1.0)
                    pkh = ptile(pctx, [128, HALO], "pctx")
                    pmac(pkh, lambda c, j=j: wk_sb[:, c, j * 128:(j + 1) * 128],
                         lambda c: hT[:, c, 0:HALO])
                    nc.vector.tensor_scalar_add(kT[:, j, 0:HALO], pkh, bk_sb[:, j:j + 1])

            # ---- attention, software-pipelined: scores run LAG ahead of
            # transpose+ctx so the PE stream never waits on the softmax chain
            # (the S->exp->sum->recip->norm chain spans ~2us of cross-engine
            # latency; at ~0.5us of PE work per iteration LAG=4 covers it).
            LAG = 4

            def s_block(qb, j2):
                """PE: mask bias matmul + 2 head score matmuls into one PSUM
                tile; ACT: exp; DVE: rowsum, reciprocal, normalize (bf16)."""
                bias_t = bias0 if qb == 0 else biasr
                ps = ptile(pscore, [128, 512], "psc")
                for hi, r in enumerate((0, 64)):
                    sl = slice(hi * 256, (hi + 1) * 256)
                    nc.tensor.matmul(ps[:, sl], ident, bias_t[:, sl],
                                     start=True, stop=False)
                    nc.tensor.matmul(ps[:, sl],
                                     qT[r:r + 64, j2, qb * 128:(qb + 1) * 128],
                                     kT[r:r + 64, j2, qb * 128:qb * 128 + 256],
                                     start=False, stop=True)
                p_pair = soft.tile([128, 512], BF16, tag="p_pair")
                nc.scalar.activation(out=p_pair, in_=ps, func=AF.Exp,
                                     bias=0.0, scale=1.0)
                rs = soft.tile([128, 2], F32, tag="rs")
                nc.vector.reduce_sum(out=rs, in_=p_pair.rearrange("p (h k) -> p h k", h=2),
                                     axis=AX.X)
                rinv = soft.tile([128, 2], F32, tag="rinv")
                nc.vector.reciprocal(rinv, rs)
                for hi in range(2):
                    nc.vector.tensor_scalar_mul(p_pair[:, hi * 256:(hi + 1) * 256],
                                                p_pair[:, hi * 256:(hi + 1) * 256],
                                                rinv[:, hi:hi + 1])
                return p_pair

            def tc_block(qb, j2, p_pair):
                """PE: transpose probs, ctx matmuls; Pool: pT evict;
                ACT: ctxT evict (fp8)."""
                ptp = ptile_bf(ptr, [128, 512], "ptr")
                for q4 in range(4):
                    nc.tensor.transpose(ptp[:, q4 * 128:(q4 + 1) * 128],
                                        p_pair[:, q4 * 128:(q4 + 1) * 128], ident)
                pT = soft.tile([128, 512], BF16, tag="pT")
                if j2 % 2 == 0:
                    nc.vector.tensor_copy(out=pT, in_=ptp)
                else:
                    nc.scalar.copy(out=pT, in_=ptp)
                pc = ptile(pctx, [128, 128], "pctx")
                for hi, r in enumerate((0, 64)):
                    h = 2 * j2 + hi
                    for half in range(2):
                        kb = qb + half
                        nc.tensor.matmul(pc[r:r + 64, :],
                                         v_sb[:, kb, h * 64:(h + 1) * 64],
                                         pT[:, (hi * 2 + half) * 128:(hi * 2 + half + 1) * 128],
                                         start=(half == 0), stop=(half == 1),
                                         tile_position=(0, r))
                nc.scalar.copy(out=ctxT[:, j2, qb * 128:(qb + 1) * 128], in_=pc)

            def oproj_now(t):
                """Out-projection + residual + LN2 stats (DVE-only start of
                the LN2 chain — the cross-engine pieces are deferred so no
                engine queue-head blocks the attention pipeline)."""
                for n in range(2):
                    po = ptile(pmm, [128, 512], "mm")
                    pmac(po, lambda c, t=t: ctxT[:, c, t * 128:(t + 1) * 128],
                         lambda c, n=n: wo_sb[:, c, n * 512:(n + 1) * 512])
                    sl = slice(n * 512, (n + 1) * 512)
                    nc.vector.tensor_add(x2_sb[:, t, sl], po, x_sb[:, t + 1, sl])
                stats = ln.tile([128, 2, 6], F32, tag="ln_stats")
                xr2 = x2_sb[:, t, :].rearrange("p (s f) -> p s f", f=512)
                for s in range(2):
                    nc.vector.bn_stats(out=stats[:, s, :], in_=xr2[:, s, :])
                mv = ln.tile([128, 2], F32, tag="ln_mv")
                nc.vector.bn_aggr(out=mv[:, :], in_=stats[:, :, :])
                return mv

            def ln2_rstd(t, mv, box):
                lnv = ln.tile([128, 1], F32, tag="ln_lnv")
                nc.scalar.activation(out=lnv, in_=mv[:, 1:2], func=AF.Ln,
                                     bias=eps_tile, scale=1.0)
                rstd = ln.tile([128, 1], F32, tag="ln_rstd")
                nc.scalar.activation(out=rstd, in_=lnv, func=AF.Exp, bias=0.0, scale=-0.5)
                box.append(rstd)

            def ln2_nmr(t, mv, box):
                rstd = box[0]
                nmr = ln.tile([128, 1], F32, tag="ln_nmr")
                nc.vector.scalar_tensor_tensor(out=nmr, in0=mv[:, 0:1], scalar=-1.0,
                                               in1=rstd, op0=mybir.AluOpType.mult,
                                               op1=mybir.AluOpType.mult)
                box.append(nmr)

            def ln2_apply(t, mv, box):
                rstd, nmr = box[0], box[1]
                h2_t = scr.tile([128, D], BF16, tag="h2_t")
                nc.scalar.activation(out=h2_t, in_=x2_sb[:, t, :], func=AF.Identity,
                                     bias=nmr, scale=rstd)
                for g in range(2):
                    pt = ptile_bf(ptr, [128, 512], "ptr")
                    for jj in range(4):
                        j = g * 4 + jj
                        nc.tensor.transpose(pt[:, jj * 128:(jj + 1) * 128],
                                            h2_t[:, j * 128:(j + 1) * 128], ident)
                    dst = h2T[:, g * 4:(g + 1) * 4, t * 128:(t + 1) * 128]
                    if (t * 2 + g) % 2 == 0:
                        nc.vector.tensor_copy(out=dst, in_=pt.rearrange("p (j c) -> p j c", j=4))
                    else:
                        nc.scalar.copy(out=dst, in_=pt.rearrange("p (j c) -> p j c", j=4))
                # final-residual bias pre-add (after LN2 consumed x2[t])
                nc.vector.tensor_add(x2_sb[:, t, :], x2_sb[:, t, :], b2_bc)

            pending = []
            deferred = []
            for it in range(32 + LAG):
                if it < 32:
                    qb, j2 = divmod(it, 8)
                    pending.append((qb, j2, s_block(qb, j2)))
                if it >= LAG:
                    qb2, j22, pp = pending.pop(0)
                    tc_block(qb2, j22, pp)
                    if j22 == 7:
                        mv = oproj_now(qb2)
                        box = []
                        deferred.extend([
                            lambda t=qb2, mv=mv, box=box: ln2_rstd(t, mv, box),
                            lambda t=qb2, mv=mv, box=box: ln2_nmr(t, mv, box),
                            lambda t=qb2, mv=mv, box=box: ln2_apply(t, mv, box),
                        ])
                    elif deferred:
                        deferred.pop(0)()
            while deferred:
                deferred.pop(0)()

        # ---- FFN (bf16) ----
        with st(name="ffnw", bufs=1) as ffnw, st(name="w1p", bufs=2) as w1p, \
             st(name="outp", bufs=2) as outp:
            gT = ffnw.tile([128, 32, SEG], BF16)
            w2_sb = ffnw.tile([128, 32, D], BF16)

            w2r = ext["w2_ext"].rearrange("(c p) n -> p c n", p=128)
            for c in range(4):
                nc.gpsimd.dma_start(out=w2_sb[:, c * 8:(c + 1) * 8, :],
                                    in_=w2r[:, c * 8:(c + 1) * 8, :])

            w1r = ext["w1_ext"].rearrange("(k p) n -> p k n", p=128)
            for c in range(4):
                w1c = w1p.tile([128, 8, 1024], BF16, tag="w1c")
                nc.gpsimd.dma_start(out=w1c, in_=w1r[:, :, c * 1024:(c + 1) * 1024])
                for jj in range(8):
                    jdff = c * 8 + jj
                    pg = ptile(pmm, [128, SEG], "mm")
                    for k in range(8):
                        nc.tensor.matmul(pg, w1c[:, k, jj * 128:(jj + 1) * 128],
                                         h2T[:, k, :], start=(k == 0), stop=(k == 7))
                    nc.scalar.activation(out=gT[:, jdff, :], in_=pg, func=AF.Gelu_apprx_tanh,
                                         bias=b1_sb[:, jdff:jdff + 1], scale=1.0)

            outr = ext["out_ext"].rearrange("(t p) d -> p t d", p=128)
            for t in range(4):
                o_t = outp.tile([128, D], F32, tag="o_t")
                for n in range(2):
                    py = ptile(pmm, [128, 512], "mm")
                    for k in range(32):
                        nc.tensor.matmul(py, gT[:, k, t * 128:(t + 1) * 128],
                                         w2_sb[:, k, n * 512:(n + 1) * 512],
                                         start=(k == 0), stop=(k == 31))
                    sl = slice(n * 512, (n + 1) * 512)
                    nc.vector.tensor_add(o_t[:, sl], py, x2_sb[:, t, sl])
                nc.sync.dma_start(out=outr[:, t, :], in_=o_t)


def _host_prep(x, Wq, bq, Wk, bk, Wv, bv, Wo, bo, W1, b1, W2, b2,
               ln1_w, ln1_b, ln2_w, ln2_b):
    bf = ml_dtypes.bfloat16
    f8 = ml_dtypes.float8_e4m3fn if _USE_FP8 else bf

    def q8(a):
        return np.ascontiguousarray(
            np.clip(np.asarray(a, np.float32), -240.0, 240.0).astype(f8))

    sc = 1.0 / np.sqrt(DH)
    wq_eff = q8((ln1_w[:, None] * Wq) * sc)
    bq_eff = ((bq + ln1_b @ Wq) * sc).astype(np.float32)
    wk_eff = q8(ln1_w[:, None] * Wk)
    bk_eff = (bk + ln1_b @ Wk).astype(np.float32)
    wv_eff = q8(ln1_w[:, None] * Wv)
    bv_eff = (bv + ln1_b @ Wv).astype(np.float32)
    w1_eff = (ln2_w[:, None] * W1).astype(bf)
    b1_eff = (b1 + ln2_b @ W1).astype(np.float32)

    r = np.arange(128)[:, None]
    c = np.arange(128)[None, :]
    left = np.where(c >= r, 0.0, NEG).astype(np.float32)
    diag = np.where(c <= r, 0.0, NEG).astype(np.float32)
    biasr = np.concatenate([left, diag, left, diag], axis=1).astype(bf)
    bias0_halo = np.concatenate(
        [np.full((128, 128), NEG, np.float32), diag,
         np.full((128, 128), NEG, np.float32), diag], axis=1).astype(bf)

    shared = {
        "wq": wq_eff, "wk": wk_eff, "wv": wv_eff,
        "wo": q8(Wo),
        "w1": w1_eff, "w2": np.ascontiguousarray(W2.astype(bf)),
        "bq": bq_eff, "bk": bk_eff, "bv": bv_eff,
        "bo": bo.astype(np.float32), "b1": b1_eff, "b2": b2.astype(np.float32),
        "biasr": biasr,
    }
    in_maps = []
    for core in range(NSEG):
        b_, s_ = core // 4, core % 4
        if s_ == 0:
            seg = np.concatenate(
                [np.zeros((HALO, D), np.float32), x[b_, 0:SEG]], axis=0)
            bias0 = bias0_halo
        else:
            seg = x[b_, s_ * SEG - HALO: (s_ + 1) * SEG]
            bias0 = biasr
        m = dict(shared)
        m["x"] = np.ascontiguousarray(seg.astype(np.float32))
        m["bias0"] = bias0
        in_maps.append(m)
    return in_maps


def kernel(**inputs):
    from concourse.bass_utils import run_bass_kernel_spmd

    if "nc" not in _CACHED:
        _CACHED["nc"] = _build()
    nc = _CACHED["nc"]

    in_maps = _host_prep(**{k: np.asarray(v) for k, v in inputs.items()})
    trace = bool(int(os.environ.get("KERNEL_TRACE", "0")))
    res = run_bass_kernel_spmd(nc, in_maps, list(range(NSEG)), trace=trace)
    kernel.last_results = res

    x = np.asarray(inputs["x"])
    out = np.empty((B, L, D), np.float32)
    for core in range(NSEG):
        b_, s_ = core // 4, core % 4
        out[b_, s_ * SEG:(s_ + 1) * SEG] = res.results[core]["out"]
    return out


# revision 49
# speedup vs baseline: 3.4172x; 3.2023x over previous
"""Causal local-window (W=128) attention block + FFN, distributed over 8 TRN2
NeuronCores with ZERO collectives.

Sharding: (B=2, L=2048) tokens are split into 8 contiguous segments of 512
tokens (4 per batch element). Each core receives its 512 owned tokens plus a
128-token left halo (zero-padded for the first segment of each batch) and
recomputes the halo's K/V locally — the sliding window (j in [i-128, i]) never
crosses more than 128 tokens back, so no cross-core communication is needed.

Key implementation choices (v2):
  - fp8(e4m3) weights + activations with MatmulPerfMode.DoubleRow for the
    QKV projections and the attention out-projection (2 K-chunks contracted
    per pass). FFN + attention internals stay bf16 (fp8 there would break
    the 2e-2 error budget; measured headroom: attn-side fp8 = 1.4e-2).
  - The additive sliding-window mask is applied on the PE: an identity
    matmul writes the mask into PSUM (start=True) and the score matmuls
    accumulate on top — no f32 DVE bias-add in the softmax path.
  - exp() runs on the Scalar engine straight from PSUM; row sums + the
    1/sum normalization run on DVE in bf16 (2x mode); the transposed-
    probability PSUM eviction runs on GpSimd; ctx eviction on Scalar.
    The attention inner loop is software-pipelined (scores run 2
    iterations ahead of transpose+ctx) so the PE never idles — TRN2 drops
    the PE clock 2x for ~3us after any idle gap.
  - LayerNorm rstd = exp(-0.5*ln(var+eps)): Ln, Exp, Identity and Copy all
    live in one activation table, so the Scalar engine never reloads
    tables inside the attention phase (a reload is 1.3us).
"""

import os
import numpy as np
import ml_dtypes

import concourse.bass as bass
import concourse.mybir as mybir
import concourse.tile as tile
from concourse.masks import make_identity
from bass_rust import ScopedClock

# ---------------------------------------------------------------------------
# Workarounds for the walrus build in this container, which accepts at most
# ONE sync-wait and ONE sync-update per instruction. Tile attaches one wait
# per out-of-date producer clock and one update per consumer engine, so any
# nontrivial Tile kernel violates this. Fix by splitting the extras onto
# standalone InstEventSemaphore instructions on the same engine: waits go
# immediately BEFORE the instruction, updates immediately AFTER (each engine
# executes its stream in order, so semantics are preserved).
_split_counter = [0]


def _split_multi_sync(nc):
    for f in nc.m.functions:
        for bb in f.blocks:
            il = list(bb.instructions)
            new = []
            changed = False
            for inst in il:
                si = inst.sync_info
                waits = list(si.on_wait) if si and si.on_wait else []
                upds = list(si.on_update) if si and si.on_update else []
                if len(waits) > 1:
                    changed = True
                    for w in waits[:-1]:
                        _split_counter[0] += 1
                        new.append(mybir.InstEventSemaphore(
                            name=f"I-wsplit-{_split_counter[0]}",
                            engine=inst.engine, ins=[], outs=[],
                            sync_info=mybir.SyncInfo(on_wait=[w], on_update=[]),
                        ))
                    si.on_wait = [waits[-1]]
                new.append(inst)
                if len(upds) > 1:
                    changed = True
                    si.on_update = [upds[0]]
                    for u in upds[1:]:
                        _split_counter[0] += 1
                        new.append(mybir.InstEventSemaphore(
                            name=f"I-usplit-{_split_counter[0]}",
                            engine=inst.engine, ins=[], outs=[],
                            sync_info=mybir.SyncInfo(on_wait=[], on_update=[u]),
                        ))
            if changed:
                bb.instructions = new


def _patched_drain_and_barrier(self, tick_clock, wait_clock):
    # Tile's kernel-tail drain carries one wait per logical processor; split
    # them into standalone single-wait SP instructions instead.
    nc = self.nc
    drain_inst = nc.sync.drain()
    wait_clock.add_sem_waits(drain_inst.ins, ScopedClock({None: tick_clock.global_clock}))
    si = drain_inst.ins.sync_info
    waits = list(si.on_wait or [])
    if len(waits) > 1:
        si.on_wait = []
        handles = {}
        for s in self.sems.allocated().values():
            nm = getattr(s, 'ant_name', None) or getattr(s, 'name', None)
            handles[nm] = s
        for w in waits:
            assert w.wait_mode == 'sem-ge-imm', w
            nc.sync.wait_ge(handles[w.ant_name], w.wait_value)
    nc.all_engine_barrier()
    assert self.sems is not None
    popped = nc._tile_sem_poison_stack.pop()
    assert popped is self._sem_poison
    nc.clear_and_free_semaphores(list(self.sems.allocated().values()))
    nc.all_engine_barrier()


tile.TileContext._drain_and_barrier = _patched_drain_and_barrier

F32 = mybir.dt.float32
BF16 = mybir.dt.bfloat16
AF = mybir.ActivationFunctionType
AX = mybir.AxisListType

# debug toggles (read at build time)
_USE_FP8 = os.environ.get("K_FP8", "0") == "1"
_USE_DR = os.environ.get("K_DR", "1") == "1" and _USE_FP8
FP8 = mybir.dt.float8e4 if _USE_FP8 else mybir.dt.bfloat16
DR = mybir.MatmulPerfMode.DoubleRow if _USE_DR else None

B, L, D = 2, 2048, 1024
NH, DH = 16, 64
DFF = 4096
WIN = 128
SEG = 512          # owned tokens per core
HALO = 128
T = SEG + HALO     # 640 local tokens
NT = T // 128      # 5 local token tiles
NSEG = 8           # cores
NEG = -1.0e30
LN_EPS = 1e-5

_CACHED = {}


def _build(split=True):
    nc = bass.Bass()
    x_ext = nc.declare_dram_parameter("x", [T, D], BF16, isOutput=False)
    wq_ext = nc.declare_dram_parameter("wq", [D, D], FP8, isOutput=False)
    wk_ext = nc.declare_dram_parameter("wk", [D, D], FP8, isOutput=False)
    wv_ext = nc.declare_dram_parameter("wv", [D, D], FP8, isOutput=False)
    wo_ext = nc.declare_dram_parameter("wo", [D, D], FP8, isOutput=False)
    w1_ext = nc.declare_dram_parameter("w1", [D, DFF], BF16, isOutput=False)
    w2_ext = nc.declare_dram_parameter("w2", [DFF, D], BF16, isOutput=False)
    bq_ext = nc.declare_dram_parameter("bq", [D], F32, isOutput=False)
    bk_ext = nc.declare_dram_parameter("bk", [D], F32, isOutput=False)
    bv_ext = nc.declare_dram_parameter("bv", [D], BF16, isOutput=False)
    bo_ext = nc.declare_dram_parameter("bo", [D], BF16, isOutput=False)
    b1_ext = nc.declare_dram_parameter("b1", [DFF], F32, isOutput=False)
    b2_ext = nc.declare_dram_parameter("b2", [D], BF16, isOutput=False)
    bias0_ext = nc.declare_dram_parameter("bias0", [128, 512], BF16, isOutput=False)
    biasr_ext = nc.declare_dram_parameter("biasr", [128, 512], BF16, isOutput=False)
    out_ext = nc.declare_dram_parameter("out", [SEG, D], F32, isOutput=True)

    with tile.TileContext(nc) as tc:
        _body(nc, tc, locals())
    if split:
        _split_multi_sync(nc)
    return nc


def _ln_stats(nc, ln, x_ap, eps_tile):
    """bn_stats + rstd via exp(-0.5*ln(var+eps)) (stays in the Exp act table).
    Returns (nmr, rstd) tiles: h = x*rstd + nmr."""
    stats = ln.tile([128, 2, 6], F32, tag="ln_stats")
    xr = x_ap.rearrange("p (s f) -> p s f", f=512)
    for s in range(2):
        nc.vector.bn_stats(out=stats[:, s, :], in_=xr[:, s, :])
    mv = ln.tile([128, 2], F32, tag="ln_mv")
    nc.vector.bn_aggr(out=mv[:, :], in_=stats[:, :, :])
    lnv = ln.tile([128, 1], F32, tag="ln_lnv")
    nc.scalar.activation(out=lnv, in_=mv[:, 1:2], func=AF.Ln, bias=eps_tile, scale=1.0)
    rstd = ln.tile([128, 1], F32, tag="ln_rstd")
    nc.scalar.activation(out=rstd, in_=lnv, func=AF.Exp, bias=0.0, scale=-0.5)
    nmr = ln.tile([128, 1], F32, tag="ln_nmr")
    # nmr = -mean * rstd in one DVE pass
    nc.vector.tensor_scalar(nmr, mv[:, 0:1], rstd, -1.0,
                            mybir.AluOpType.mult, mybir.AluOpType.mult)
    return nmr, rstd


def _body(nc, tc, ext):
    st = tc.tile_pool  # shorthand

    with (
        st(name="const", bufs=1) as const,
        st(name="resid", bufs=1) as resid,
        st(name="ln", bufs=3) as ln,
        st(name="pmm", bufs=2, space="PSUM") as pmm,
        st(name="pscore", bufs=2, space="PSUM") as pscore,
        st(name="ptr", bufs=2, space="PSUM") as ptr,
        st(name="pctx", bufs=2, space="PSUM") as pctx,
    ):
        def pmac(out, a_fn, b_fn, nk=8):
            """Accumulating K-chunk matmul: paired chunks under DoubleRow,
            per-chunk otherwise. a_fn/b_fn map a chunk slice to the operand AP."""
            if DR is not None:
                for c in range(0, nk, 2):
                    nc.tensor.matmul(out, a_fn(slice(c, c + 2)), b_fn(slice(c, c + 2)),
                                     start=(c == 0), stop=(c == nk - 2), perf_mode=DR)
            else:
                for c in range(nk):
                    nc.tensor.matmul(out, a_fn(c), b_fn(c),
                                     start=(c == 0), stop=(c == nk - 1))

        def ptile(pool, shape, tg):
            return pool.tile(shape, F32, tag=tg, name="pst_" + tg)

        def ptile_bf(pool, shape, tg):
            return pool.tile(shape, BF16, tag=tg, name="pstb_" + tg)

        # ---- constants. All input DMAs go through the sync queue in
        # priority order (x first, then wv/wq/wk/wo in consumption order) so
        # the front-end is never waiting on a transfer that queued late.
        ident = const.tile([128, 128], BF16)
        make_identity(nc, ident)
        ones_col = const.tile([128, 1], BF16)
        nc.vector.memset(ones_col, 1.0)
        ones_row = const.tile([1, 128], BF16)
        nc.vector.memset(ones_row, 1.0)
        x_sb = const.tile([128, NT, D], BF16)
        xr = ext["x_ext"].rearrange("(t p) d -> p t d", p=128)
        for t in range(NT):
            nc.sync.dma_start(out=x_sb[:, t, :], in_=xr[:, t, :])
        eps_tile = const.tile([128, 1], F32)
        nc.vector.memset(eps_tile, LN_EPS)

        def bcast(name):
            t_ = const.tile([128, D], BF16, tag=f"bc_{name}")
            src = ext[f"{name}_ext"][:]
            ap = bass.AP(tensor=src.tensor, offset=src.offset,
                         ap=[[0, 128]] + list(src.ap))
            nc.gpsimd.dma_start(out=t_, in_=ap)
            return t_

        bv_bc = bcast("bv")
        bo_bc = bcast("bo")
        b2_bc = bcast("b2")
        bq_sb = const.tile([128, 8], F32)
        nc.gpsimd.dma_start(out=bq_sb, in_=ext["bq_ext"].rearrange("(j p) -> p j", p=128))
        bk_sb = const.tile([128, 8], F32)
        nc.gpsimd.dma_start(out=bk_sb, in_=ext["bk_ext"].rearrange("(j p) -> p j", p=128))
        b1_sb = const.tile([128, 32], F32)
        nc.gpsimd.dma_start(out=b1_sb, in_=ext["b1_ext"].rearrange("(j p) -> p j", p=128))
        bias0 = const.tile([128, 512], BF16)
        nc.gpsimd.dma_start(out=bias0, in_=ext["bias0_ext"][:, :])
        biasr = const.tile([128, 512], BF16)
        nc.gpsimd.dma_start(out=biasr, in_=ext["biasr_ext"][:, :])

        x2_sb = resid.tile([128, 4, D], F32)
        h2T = resid.tile([128, 8, SEG], BF16)

        # ---- PE warmup: prime the p-state ramp while x loads (PE must stay
        # busy ~3us continuously to reach full clock, and idle gaps re-throttle
        # it to half clock) ----
        for i in range(24):
            wu = ptile_bf(ptr, [128, 512], "ptr")
            for q4 in range(4):
                nc.tensor.transpose(wu[:, q4 * 128:(q4 + 1) * 128], ident, ident)

        with st(name="attnw", bufs=1) as attnw, st(name="scr", bufs=3) as scr, \
             st(name="soft", bufs=5) as soft:
            wo_sb = attnw.tile([128, 8, D], FP8)
            qT = attnw.tile([128, 8, SEG], BF16)
            kT = attnw.tile([128, 8, T], BF16)
            v_sb = attnw.tile([128, NT, D], BF16)
            ctxT = attnw.tile([128, 8, SEG], FP8)

            with st(name="qkvw", bufs=1) as qkvw:
                wv_sb = qkvw.tile([128, 8, D], FP8)
                nc.gpsimd.dma_start(out=wv_sb, in_=ext["wv_ext"].rearrange("(k p) n -> p k n", p=128))
                wq_sb = qkvw.tile([128, 8, D], FP8)
                nc.gpsimd.dma_start(out=wq_sb, in_=ext["wq_ext"].rearrange("(k p) n -> p k n", p=128))
                wk_sb = qkvw.tile([128, 8, D], FP8)
                nc.gpsimd.dma_start(out=wk_sb, in_=ext["wk_ext"].rearrange("(k p) n -> p k n", p=128))
                nc.gpsimd.dma_start(out=wo_sb, in_=ext["wo_ext"].rearrange("(k p) n -> p k n", p=128))
                hT = qkvw.tile([128, 8, T], FP8)

                # ---- LN1: stats stream in as x tiles land (DVE only, no
                # cross-engine ping-pong), then ONE batched rstd for all 5
                # tiles, then apply/transpose/V per tile.
                mv_all = qkvw.tile([128, NT, 2], F32)
                for t in range(NT):
                    stats = ln.tile([128, 2, 6], F32, tag="ln_stats")
                    xr = x_sb[:, t, :].rearrange("p (s f) -> p s f", f=512)
                    for s in range(2):
                        nc.vector.bn_stats(out=stats[:, s, :], in_=xr[:, s, :])
                    nc.vector.bn_aggr(out=mv_all[:, t, :], in_=stats[:, :, :])
                lnv5 = qkvw.tile([128, NT], F32)
                nc.scalar.activation(out=lnv5, in_=mv_all[:, :, 1], func=AF.Ln,
                                     bias=eps_tile, scale=1.0)
                rstd5 = qkvw.tile([128, NT], F32)
                nc.scalar.activation(out=rstd5, in_=lnv5, func=AF.Exp, bias=0.0, scale=-0.5)
                nmr5 = qkvw.tile([128, NT], F32)
                nc.vector.scalar_tensor_tensor(out=nmr5, in0=mv_all[:, :, 0], scalar=-1.0,
                                               in1=rstd5, op0=mybir.AluOpType.mult,
                                               op1=mybir.AluOpType.mult)
                for t in range(NT):
                    h_t = scr.tile([128, D], BF16, tag="h_t")
                    nc.scalar.activation(out=h_t, in_=x_sb[:, t, :], func=AF.Identity,
                                         bias=nmr5[:, t:t + 1], scale=rstd5[:, t:t + 1])
                    for g in range(2):
                        pt = ptile_bf(ptr, [128, 512], "ptr")
                        for jj in range(4):
                            j = g * 4 + jj
                            nc.tensor.transpose(pt[:, jj * 128:(jj + 1) * 128],
                                                h_t[:, j * 128:(j + 1) * 128], ident)
                        dst = hT[:, g * 4:(g + 1) * 4, t * 128:(t + 1) * 128]
                        if (t * 2 + g) % 2 == 0:
                            nc.vector.tensor_copy(out=dst, in_=pt.rearrange("p (j c) -> p j c", j=4))
                        else:
                            nc.scalar.copy(out=dst, in_=pt.rearrange("p (j c) -> p j c", j=4))
                    # V for this tile: lhsT = hT chunk (pairs under DoubleRow)
                    for n in range(2):
                        pv = ptile(pmm, [128, 512], "mm")
                        pmac(pv, lambda c, t=t: hT[:, c, t * 128:(t + 1) * 128],
                             lambda c, n=n: wv_sb[:, c, n * 512:(n + 1) * 512])
                        nc.vector.tensor_add(v_sb[:, t, n * 512:(n + 1) * 512], pv,
                                             bv_bc[:, n * 512:(n + 1) * 512])

                # residual bias pre-add (off critical path; ordered after LN1 reads)
                for t in range(4):
                    nc.vector.tensor_add(x_sb[:, t + 1, :], x_sb[:, t + 1, :], bo_bc)

                # ---- Q/K projections (DoubleRow fp8) ----
                for j in range(8):
                    pq = ptile(pmm, [128, SEG], "mm")
                    pmac(pq, lambda c, j=j: wq_sb[:, c, j * 128:(j + 1) * 128],
                         lambda c: hT[:, c, HALO:T])
                    nc.scalar.activation(out=qT[:, j, :], in_=pq, func=AF.Identity,
                                         bias=bq_sb[:, j:j + 1], scale=1.0)
                for j in range(8):
                    pk = ptile(pmm, [128, SEG], "mm")
                    pmac(pk, lambda c, j=j: wk_sb[:, c, j * 128:(j + 1) * 128],
                         lambda c: hT[:, c, HALO:T])
                    nc.scalar.activation(out=kT[:, j, HALO:T], in_=pk, func=AF.Identity,
                                         bias=bk_sb[:, j:j + 1], scale=1.0)
                    pkh = ptile(pmm, [128, 512], "mm")
                    pmac(pkh[:, 0:HALO], lambda c, j=j: wk_sb[:, c, j * 128:(j + 1) * 128],
                         lambda c: hT[:, c, 0:HALO])
                    nc.vector.tensor_scalar_add(kT[:, j, 0:HALO], pkh[:, 0:HALO], bk_sb[:, j:j + 1])

            if os.environ.get("K_FRONTONLY", "0") == "1":
                if os.environ.get("K_FRONTPLUS", "0") == "1":
                    ps0 = ptile(pscore, [128, 512], "psc")
                    nc.tensor.matmul(ps0[:, 0:256],
                                     qT[0:64, 0, 0:128], kT[0:64, 0, 0:256],
                                     start=True, stop=True)
                    pe0 = soft.tile([128, 512], BF16, tag="pE")
                    nc.vector.tensor_copy(out=pe0, in_=ps0)
                outr0 = ext["out_ext"].rearrange("(t p) d -> p t d", p=128)
                for t in range(4):
                    o0 = scr.tile([128, D], F32, tag="h_t")
                    nc.vector.tensor_copy(out=o0, in_=v_sb[:, t, :])
                    nc.sync.dma_start(out=outr0[:, t, :], in_=o0)
                return

            # ---- attention. Scores are computed TRANSPOSED (sT[keys, q]):
            # the exp() activation evicts PSUM straight into the probability
            # tile pT (no PE transpose, no separate eviction), per-head key
            # sums come from a ones-column matmul on the PE, and the 1/sum
            # normalization is folded into the ctx PSUM eviction via a
            # PE-broadcast reciprocal tile. Two-slot software pipeline so the
            # PE never waits on the exp/recip chain.
            def s_block(qb, j2):
                """PE: 4 transposed-score matmuls sT[keys,q]; ACT: exp straight
                to SBUF pT; DVE: multiplicative 0/1 window mask (masked exp
                values are garbage but finite — the mask zeroes them before
                the sums/ctx matmuls read pT). Block b = kt*2+hi."""
                mask_t = bias0 if qb == 0 else biasr
                ps = ptile(pmm if os.environ.get("K_SCOREPOOL") == "mm" else pscore,
                           [128, 512], "mm" if os.environ.get("K_SCOREPOOL") == "mm" else "psc")
                if os.environ.get("K_OLDSCORE", "0") == "1":
                    for hi, r in enumerate((0, 64)):
                        nc.tensor.matmul(ps[:, hi * 256:(hi + 1) * 256],
                                         qT[r:r + 64, j2, qb * 128:(qb + 1) * 128],
                                         kT[r:r + 64, j2, qb * 128:qb * 128 + 256],
                                         start=True, stop=True)
                else:
                    for kt in range(2):
                        for hi, r in enumerate((0, 64)):
                            b = kt * 2 + hi
                            nc.tensor.matmul(ps[:, b * 128:(b + 1) * 128],
                                             kT[r:r + 64, j2, (qb + kt) * 128:(qb + kt + 1) * 128],
                                             qT[r:r + 64, j2, qb * 128:(qb + 1) * 128],
                                             start=True, stop=True)
                pE = soft.tile([128, 512], BF16, tag="pE")
                _LVL = int(os.environ.get("K_ATTLVL", "9"))
                if _LVL == 0:
                    nc.vector.tensor_copy(out=pE, in_=ps)
                else:
                    nc.scalar.activation(out=pE, in_=ps, func=AF.Exp, bias=0.0, scale=1.0)
                if _LVL <= 1 and _LVL >= 0 and os.environ.get("K_NOMASK", "0") == "1":
                    return pE
                pT = soft.tile([128, 512], BF16, tag="pT")
                nc.vector.tensor_mul(pT, pE, mask_t)
                return pT

            _HANGTEST = os.environ.get("K_HANGTEST", "0") == "1"

            def sums_block(qb, j2, pT):
                if _HANGTEST:
                    return None, None
                """PE: per-head key-sums (ones-column matmul over both key
                tiles; head sums land at PSUM partitions 0 and 64 — matmul
                outputs must be quadrant-aligned); DVE: reciprocal into a
                [2,128] bf16 row tile."""
                # one PSUM tile bundles ctx (cols 0:128), key-sums (128:256)
                # and the broadcast reciprocal (256:384) -- PSUM pools round
                # every tag up to a full bank, so split tiles would not fit.
                bundle = ptile(pctx, [128, 384], "sb")
                rs = bundle[:, 128:256]
                for hi in range(2):
                    for kt in range(2):
                        b = kt * 2 + hi
                        nc.tensor.matmul(rs[hi * 64:hi * 64 + 1, :], ones_col,
                                         pT[:, b * 128:(b + 1) * 128],
                                         start=(kt == 0), stop=(kt == 1),
                                         tile_position=(0, hi * 64))
                rinv_a = soft.tile([1, 128], BF16, tag="rinva")
                rinv_b = soft.tile([1, 128], BF16, tag="rinvb")
                with nc.allow_low_precision(reason="softmax 1/sum in bf16"):
                    nc.vector.reciprocal(rinv_a, rs[0:1, :])
                    nc.vector.reciprocal(rinv_b, rs[64:65, :])
                return bundle, (rinv_a, rinv_b)

            def ctx_block(qb, j2, pT, bundle, rinv):
                """PE: broadcast rinv to [128,q] + ctx matmuls; DVE: normalize
                during the ctx PSUM eviction."""
                if _HANGTEST:
                    pc = ptile(pscore, [128, 512], "psc")[:, 0:128]
                    for hi, r in enumerate((0, 64)):
                        h = 2 * j2 + hi
                        for kt in range(2):
                            b = kt * 2 + hi
                            nc.tensor.matmul(pc[r:r + 64, :],
                                             v_sb[:, qb + kt, h * 64:(h + 1) * 64],
                                             pT[:, b * 128:(b + 1) * 128],
                                             start=(kt == 0), stop=(kt == 1),
                                             tile_position=(0, r))
                    nc.scalar.copy(out=ctxT[:, j2, qb * 128:(qb + 1) * 128], in_=pc)
                    return
                rinv_a, rinv_b = rinv
                rb = bundle[:, 256:384]
                nc.tensor.matmul(rb[0:64, :], ones_row[:, 0:64], rinv_a,
                                 start=True, stop=True, tile_position=(0, 0))
                nc.tensor.matmul(rb[64:128, :], ones_row[:, 0:64], rinv_b,
                                 start=True, stop=True, tile_position=(0, 64))
                pc = bundle[:, 0:128]
                for hi, r in enumerate((0, 64)):
                    h = 2 * j2 + hi
                    for kt in range(2):
                        b = kt * 2 + hi
                        nc.tensor.matmul(pc[r:r + 64, :],
                                         v_sb[:, qb + kt, h * 64:(h + 1) * 64],
                                         pT[:, b * 128:(b + 1) * 128],
                                         start=(kt == 0), stop=(kt == 1),
                                         tile_position=(0, r))
                # DVE may read only one PSUM operand: stage raw ctx to SBUF
                # on the Scalar engine, then normalize against the PSUM
                # broadcast tile on DVE.
                craw = soft.tile([128, 128], BF16, tag="craw")
                nc.scalar.copy(out=craw, in_=pc)
                nc.vector.tensor_mul(ctxT[:, j2, qb * 128:(qb + 1) * 128], craw, rb)

            def oproj_now(t):
                """Out-projection + residual + LN2 stats (DVE-only start of
                the LN2 chain — the cross-engine pieces are deferred so no
                engine queue-head blocks the attention pipeline)."""
                for n in range(2):
                    po = ptile(pmm, [128, 512], "mm")
                    pmac(po, lambda c, t=t: ctxT[:, c, t * 128:(t + 1) * 128],
                         lambda c, n=n: wo_sb[:, c, n * 512:(n + 1) * 512])
                    sl = slice(n * 512, (n + 1) * 512)
                    nc.vector.tensor_add(x2_sb[:, t, sl], po, x_sb[:, t + 1, sl])
                stats = ln.tile([128, 2, 6], F32, tag="ln_stats")
                xr2 = x2_sb[:, t, :].rearrange("p (s f) -> p s f", f=512)
                for s in range(2):
                    nc.vector.bn_stats(out=stats[:, s, :], in_=xr2[:, s, :])
                mv = ln.tile([128, 2], F32, tag="ln_mv")
                nc.vector.bn_aggr(out=mv[:, :], in_=stats[:, :, :])
                return mv

            def ln2_rstd(t, mv, box):
                lnv = ln.tile([128, 1], F32, tag="ln_lnv")
                nc.scalar.activation(out=lnv, in_=mv[:, 1:2], func=AF.Ln,
                                     bias=eps_tile, scale=1.0)
                rstd = ln.tile([128, 1], F32, tag="ln_rstd")
                nc.scalar.activation(out=rstd, in_=lnv, func=AF.Exp, bias=0.0, scale=-0.5)
                box.append(rstd)

            def ln2_nmr(t, mv, box):
                rstd = box[0]
                nmr = ln.tile([128, 1], F32, tag="ln_nmr")
                nc.vector.scalar_tensor_tensor(out=nmr, in0=mv[:, 0:1], scalar=-1.0,
                                               in1=rstd, op0=mybir.AluOpType.mult,
                                               op1=mybir.AluOpType.mult)
                box.append(nmr)

            def ln2_apply(t, mv, box):
                rstd, nmr = box[0], box[1]
                h2_t = scr.tile([128, D], BF16, tag="h2_t")
                nc.scalar.activation(out=h2_t, in_=x2_sb[:, t, :], func=AF.Identity,
                                     bias=nmr, scale=rstd)
                for g in range(2):
                    pt = ptile_bf(ptr, [128, 512], "ptr")
                    for jj in range(4):
                        j = g * 4 + jj
                        nc.tensor.transpose(pt[:, jj * 128:(jj + 1) * 128],
                                            h2_t[:, j * 128:(j + 1) * 128], ident)
                    dst = h2T[:, g * 4:(g + 1) * 4, t * 128:(t + 1) * 128]
                    if (t * 2 + g) % 2 == 0:
                        nc.vector.tensor_copy(out=dst, in_=pt.rearrange("p (j c) -> p j c", j=4))
                    else:
                        nc.scalar.copy(out=dst, in_=pt.rearrange("p (j c) -> p j c", j=4))
                # final-residual bias pre-add (after LN2 consumed x2[t])
                nc.vector.tensor_add(x2_sb[:, t, :], x2_sb[:, t, :], b2_bc)

            _ATTLVL = int(os.environ.get("K_ATTLVL", "9"))
            if _ATTLVL == 0:
                _ATTLVL = 1
            if _ATTLVL == 1:
                # scores+exp+mask only
                keep = []
                for it in range(int(os.environ.get("K_NITER", "32"))):
                    qb, j2 = divmod(it, 8)
                    keep.append(s_block(qb, j2))
                outr1 = ext["out_ext"].rearrange("(t p) d -> p t d", p=128)
                for t in range(4):
                    o1 = scr.tile([128, D], F32, tag="h_t")
                    nc.vector.tensor_copy(out=o1[:, 0:512], in_=keep[-1])
                    nc.vector.tensor_copy(out=o1[:, 512:1024], in_=keep[len(keep) >= 2 and -2 or -1])
                    nc.sync.dma_start(out=outr1[:, t, :], in_=o1)
                return

            pend1 = []   # awaiting sums (lag 1)
            pend2 = []   # awaiting ctx (lag 2)
            deferred = []
            for it in range(32 + 2):
                if it < 32:
                    qb, j2 = divmod(it, 8)
                    pend1.append((qb, j2, s_block(qb, j2)))
                if pend1 and (len(pend1) > 1 or it >= 32):
                    qb1, j21, pT1 = pend1.pop(0)
                    bundle1, rinv1 = sums_block(qb1, j21, pT1)
                    pend2.append((qb1, j21, pT1, bundle1, rinv1))
                if pend2 and (len(pend2) > 1 or it >= 32):
                    qb2, j22, pT2, bundle2, rinv2 = pend2.pop(0)
                    ctx_block(qb2, j22, pT2, bundle2, rinv2)
                    if j22 == 7:
                        mv = oproj_now(qb2)
                        box = []
                        deferred.extend([
                            lambda t=qb2, mv=mv, box=box: ln2_rstd(t, mv, box),
                            lambda t=qb2, mv=mv, box=box: ln2_nmr(t, mv, box),
                            lambda t=qb2, mv=mv, box=box: ln2_apply(t, mv, box),
                        ])
                    elif deferred:
                        deferred.pop(0)()
            while deferred:
                deferred.pop(0)()

        # ---- FFN (bf16) ----
        with st(name="ffnw", bufs=1) as ffnw, st(name="w1p", bufs=2) as w1p, \
             st(name="outp", bufs=2) as outp:
            gT = ffnw.tile([128, 32, SEG], BF16)
            w2_sb = ffnw.tile([128, 32, D], BF16)

            w2r = ext["w2_ext"].rearrange("(c p) n -> p c n", p=128)
            for c in range(4):
                nc.gpsimd.dma_start(out=w2_sb[:, c * 8:(c + 1) * 8, :],
                                    in_=w2r[:, c * 8:(c + 1) * 8, :])

            w1r = ext["w1_ext"].rearrange("(k p) n -> p k n", p=128)
            for c in range(4):
                w1c = w1p.tile([128, 8, 1024], BF16, tag="w1c")
                nc.gpsimd.dma_start(out=w1c, in_=w1r[:, :, c * 1024:(c + 1) * 1024])
                for jj in range(8):
                    jdff = c * 8 + jj
                    pg = ptile(pmm, [128, SEG], "mm")
                    for k in range(8):
                        nc.tensor.matmul(pg, w1c[:, k, jj * 128:(jj + 1) * 128],
                                         h2T[:, k, :], start=(k == 0), stop=(k == 7))
                    nc.scalar.activation(out=gT[:, jdff, :], in_=pg, func=AF.Gelu_apprx_tanh,
                                         bias=b1_sb[:, jdff:jdff + 1], scale=1.0)

            outr = ext["out_ext"].rearrange("(t p) d -> p t d", p=128)
            for t in range(4):
                o_t = outp.tile([128, D], F32, tag="o_t")
                for n in range(2):
                    py = ptile(pmm, [128, 512], "mm")
                    for k in range(32):
                        nc.tensor.matmul(py, gT[:, k, t * 128:(t + 1) * 128],
                                         w2_sb[:, k, n * 512:(n + 1) * 512],
                                         start=(k == 0), stop=(k == 31))
                    sl = slice(n * 512, (n + 1) * 512)
                    nc.vector.tensor_add(o_t[:, sl], py, x2_sb[:, t, sl])
                nc.sync.dma_start(out=outr[:, t, :], in_=o_t)


def _host_prep(x, Wq, bq, Wk, bk, Wv, bv, Wo, bo, W1, b1, W2, b2,
               ln1_w, ln1_b, ln2_w, ln2_b):
    bf = ml_dtypes.bfloat16
    f8 = ml_dtypes.float8_e4m3fn if _USE_FP8 else bf

    def q8(a):
        return np.ascontiguousarray(
            np.clip(np.asarray(a, np.float32), -240.0, 240.0).astype(f8))

    sc = 1.0 / np.sqrt(DH)
    wq_eff = q8((ln1_w[:, None] * Wq) * sc)
    bq_eff = ((bq + ln1_b @ Wq) * sc).astype(np.float32)
    wk_eff = q8(ln1_w[:, None] * Wk)
    bk_eff = (bk + ln1_b @ Wk).astype(np.float32)
    wv_eff = q8(ln1_w[:, None] * Wv)
    bv_eff = (bv + ln1_b @ Wv).astype(np.float32)
    w1_eff = (ln2_w[:, None] * W1).astype(bf)
    b1_eff = (b1 + ln2_b @ W1).astype(np.float32)

    # Transposed masks for the sT-layout scores: block b = kt*2 + hi holds
    # sT[keys(tile kt), q] for head hi, so the mask tile is the transpose of
    # the [q, keys] mask, duplicated for the two heads of a pair.
    r = np.arange(128)[:, None]
    c = np.arange(128)[None, :]
    left = np.where(c >= r, 1.0, 0.0).astype(np.float32)
    diag = np.where(c <= r, 1.0, 0.0).astype(np.float32)
    fullzero = np.zeros((128, 128), np.float32)
    m0T, m1T = np.ascontiguousarray(left.T), np.ascontiguousarray(diag.T)
    biasr = np.concatenate([m0T, m0T, m1T, m1T], axis=1).astype(bf)
    bias0_halo = np.concatenate([fullzero, fullzero, m1T, m1T], axis=1).astype(bf)

    shared = {
        "wq": wq_eff, "wk": wk_eff, "wv": wv_eff,
        "wo": q8(Wo),
        "w1": w1_eff, "w2": np.ascontiguousarray(W2.astype(bf)),
        "bq": bq_eff, "bk": bk_eff, "bv": bv_eff.astype(bf),
        "bo": bo.astype(bf), "b1": b1_eff, "b2": b2.astype(bf),
        "biasr": biasr,
    }
    in_maps = []
    for core in range(NSEG):
        b_, s_ = core // 4, core % 4
        if s_ == 0:
            seg = np.concatenate(
                [np.zeros((HALO, D), np.float32), x[b_, 0:SEG]], axis=0)
            bias0 = bias0_halo
        else:
            seg = x[b_, s_ * SEG - HALO: (s_ + 1) * SEG]
            bias0 = biasr
        m = dict(shared)
        m["x"] = np.ascontiguousarray(seg.astype(bf))
        m["bias0"] = bias0
        in_maps.append(m)
    return in_maps


def kernel(**inputs):
    from concourse.bass_utils import run_bass_kernel_spmd

    if "nc" not in _CACHED:
        _CACHED["nc"] = _build()
    nc = _CACHED["nc"]

    in_maps = _host_prep(**{k: np.asarray(v) for k, v in inputs.items()})
    trace = bool(int(os.environ.get("KERNEL_TRACE", "0")))
    res = run_bass_kernel_spmd(nc, in_maps, list(range(NSEG)), trace=trace)
    kernel.last_results = res

    x = np.asarray(inputs["x"])
    out = np.empty((B, L, D), np.float32)
    for core in range(NSEG):
        b_, s_ = core // 4, core % 4
        out[b_, s_ * SEG:(s_ + 1) * SEG] = res.results[core]["out"]
    return out


# revision 52
# speedup vs baseline: 3.5562x; 1.0407x over previous
"""Causal local-window (W=128) attention block + FFN, distributed over 8 TRN2
NeuronCores with ZERO collectives.

Sharding: (B=2, L=2048) tokens are split into 8 contiguous segments of 512
tokens (4 per batch element). Each core receives its 512 owned tokens plus a
128-token left halo (zero-padded for the first segment of each batch) and
recomputes the halo's K/V locally — the sliding window (j in [i-128, i]) never
crosses more than 128 tokens back, so no cross-core communication is needed.

Key implementation choices (v2):
  - fp8(e4m3) weights + activations with MatmulPerfMode.DoubleRow for the
    QKV projections and the attention out-projection (2 K-chunks contracted
    per pass). FFN + attention internals stay bf16 (fp8 there would break
    the 2e-2 error budget; measured headroom: attn-side fp8 = 1.4e-2).
  - The additive sliding-window mask is applied on the PE: an identity
    matmul writes the mask into PSUM (start=True) and the score matmuls
    accumulate on top — no f32 DVE bias-add in the softmax path.
  - exp() runs on the Scalar engine straight from PSUM; row sums + the
    1/sum normalization run on DVE in bf16 (2x mode); the transposed-
    probability PSUM eviction runs on GpSimd; ctx eviction on Scalar.
    The attention inner loop is software-pipelined (scores run 2
    iterations ahead of transpose+ctx) so the PE never idles — TRN2 drops
    the PE clock 2x for ~3us after any idle gap.
  - LayerNorm rstd = exp(-0.5*ln(var+eps)): Ln, Exp, Identity and Copy all
    live in one activation table, so the Scalar engine never reloads
    tables inside the attention phase (a reload is 1.3us).
"""

import os
import numpy as np
import ml_dtypes

import concourse.bass as bass
import concourse.mybir as mybir
import concourse.tile as tile
from concourse.masks import make_identity
from bass_rust import ScopedClock

# ---------------------------------------------------------------------------
# Workarounds for the walrus build in this container, which accepts at most
# ONE sync-wait and ONE sync-update per instruction. Tile attaches one wait
# per out-of-date producer clock and one update per consumer engine, so any
# nontrivial Tile kernel violates this. Fix by splitting the extras onto
# standalone InstEventSemaphore instructions on the same engine: waits go
# immediately BEFORE the instruction, updates immediately AFTER (each engine
# executes its stream in order, so semantics are preserved).
_split_counter = [0]


def _split_multi_sync(nc):
    for f in nc.m.functions:
        for bb in f.blocks:
            il = list(bb.instructions)
            new = []
            changed = False
            for inst in il:
                si = inst.sync_info
                waits = list(si.on_wait) if si and si.on_wait else []
                upds = list(si.on_update) if si and si.on_update else []
                if len(waits) > 1:
                    changed = True
                    for w in waits[:-1]:
                        _split_counter[0] += 1
                        new.append(mybir.InstEventSemaphore(
                            name=f"I-wsplit-{_split_counter[0]}",
                            engine=inst.engine, ins=[], outs=[],
                            sync_info=mybir.SyncInfo(on_wait=[w], on_update=[]),
                        ))
                    si.on_wait = [waits[-1]]
                new.append(inst)
                if len(upds) > 1:
                    changed = True
                    si.on_update = [upds[0]]
                    for u in upds[1:]:
                        _split_counter[0] += 1
                        new.append(mybir.InstEventSemaphore(
                            name=f"I-usplit-{_split_counter[0]}",
                            engine=inst.engine, ins=[], outs=[],
                            sync_info=mybir.SyncInfo(on_wait=[], on_update=[u]),
                        ))
            if changed:
                bb.instructions = new


def _patched_drain_and_barrier(self, tick_clock, wait_clock):
    # Tile's kernel-tail drain carries one wait per logical processor; split
    # them into standalone single-wait SP instructions instead.
    nc = self.nc
    drain_inst = nc.sync.drain()
    wait_clock.add_sem_waits(drain_inst.ins, ScopedClock({None: tick_clock.global_clock}))
    si = drain_inst.ins.sync_info
    waits = list(si.on_wait or [])
    if len(waits) > 1:
        si.on_wait = []
        handles = {}
        for s in self.sems.allocated().values():
            nm = getattr(s, 'ant_name', None) or getattr(s, 'name', None)
            handles[nm] = s
        for w in waits:
            assert w.wait_mode == 'sem-ge-imm', w
            nc.sync.wait_ge(handles[w.ant_name], w.wait_value)
    nc.all_engine_barrier()
    assert self.sems is not None
    popped = nc._tile_sem_poison_stack.pop()
    assert popped is self._sem_poison
    nc.clear_and_free_semaphores(list(self.sems.allocated().values()))
    nc.all_engine_barrier()


tile.TileContext._drain_and_barrier = _patched_drain_and_barrier

F32 = mybir.dt.float32
BF16 = mybir.dt.bfloat16
AF = mybir.ActivationFunctionType
AX = mybir.AxisListType

# debug toggles (read at build time)
_USE_FP8 = os.environ.get("K_FP8", "0") == "1"
_USE_DR = os.environ.get("K_DR", "1") == "1" and _USE_FP8
FP8 = mybir.dt.float8e4 if _USE_FP8 else mybir.dt.bfloat16
DR = mybir.MatmulPerfMode.DoubleRow if _USE_DR else None

B, L, D = 2, 2048, 1024
NH, DH = 16, 64
DFF = 4096
WIN = 128
SEG = 512          # owned tokens per core
HALO = 128
T = SEG + HALO     # 640 local tokens
NT = T // 128      # 5 local token tiles
NSEG = 8           # cores
NEG = -1.0e30
LN_EPS = 1e-5

_CACHED = {}


def _build(split=True):
    nc = bass.Bass()
    x_ext = nc.declare_dram_parameter("x", [T, D], BF16, isOutput=False)
    wq_ext = nc.declare_dram_parameter("wq", [D, D], FP8, isOutput=False)
    wk_ext = nc.declare_dram_parameter("wk", [D, D], FP8, isOutput=False)
    wv_ext = nc.declare_dram_parameter("wv", [D, D], FP8, isOutput=False)
    wo_ext = nc.declare_dram_parameter("wo", [D, D], FP8, isOutput=False)
    w1_ext = nc.declare_dram_parameter("w1", [D, DFF], BF16, isOutput=False)
    w2_ext = nc.declare_dram_parameter("w2", [DFF, D], BF16, isOutput=False)
    bq_ext = nc.declare_dram_parameter("bq", [D], F32, isOutput=False)
    bk_ext = nc.declare_dram_parameter("bk", [D], F32, isOutput=False)
    bv_ext = nc.declare_dram_parameter("bv", [D], BF16, isOutput=False)
    bo_ext = nc.declare_dram_parameter("bo", [D], BF16, isOutput=False)
    b1_ext = nc.declare_dram_parameter("b1", [DFF], F32, isOutput=False)
    b2_ext = nc.declare_dram_parameter("b2", [D], BF16, isOutput=False)
    bias0_ext = nc.declare_dram_parameter("bias0", [128, 512], BF16, isOutput=False)
    biasr_ext = nc.declare_dram_parameter("biasr", [128, 512], BF16, isOutput=False)
    out_ext = nc.declare_dram_parameter("out", [SEG, D], F32, isOutput=True)

    with tile.TileContext(nc) as tc:
        _body(nc, tc, locals())
    if split:
        _split_multi_sync(nc)
    return nc


def _ln_stats(nc, ln, x_ap, eps_tile):
    """bn_stats + rstd via exp(-0.5*ln(var+eps)) (stays in the Exp act table).
    Returns (nmr, rstd) tiles: h = x*rstd + nmr."""
    stats = ln.tile([128, 2, 6], F32, tag="ln_stats")
    xr = x_ap.rearrange("p (s f) -> p s f", f=512)
    for s in range(2):
        nc.vector.bn_stats(out=stats[:, s, :], in_=xr[:, s, :])
    mv = ln.tile([128, 2], F32, tag="ln_mv")
    nc.vector.bn_aggr(out=mv[:, :], in_=stats[:, :, :])
    lnv = ln.tile([128, 1], F32, tag="ln_lnv")
    nc.scalar.activation(out=lnv, in_=mv[:, 1:2], func=AF.Ln, bias=eps_tile, scale=1.0)
    rstd = ln.tile([128, 1], F32, tag="ln_rstd")
    nc.scalar.activation(out=rstd, in_=lnv, func=AF.Exp, bias=0.0, scale=-0.5)
    nmr = ln.tile([128, 1], F32, tag="ln_nmr")
    # nmr = -mean * rstd in one DVE pass
    nc.vector.tensor_scalar(nmr, mv[:, 0:1], rstd, -1.0,
                            mybir.AluOpType.mult, mybir.AluOpType.mult)
    return nmr, rstd


def _body(nc, tc, ext):
    st = tc.tile_pool  # shorthand

    with (
        st(name="const", bufs=1) as const,
        st(name="resid", bufs=1) as resid,
        st(name="ln", bufs=3) as ln,
        st(name="pmm", bufs=2, space="PSUM") as pmm,
        st(name="pscore", bufs=2, space="PSUM") as pscore,
        st(name="ptr", bufs=2, space="PSUM") as ptr,
        st(name="pctx", bufs=2, space="PSUM") as pctx,
    ):
        def pmac(out, a_fn, b_fn, nk=8):
            """Accumulating K-chunk matmul: paired chunks under DoubleRow,
            per-chunk otherwise. a_fn/b_fn map a chunk slice to the operand AP."""
            if DR is not None:
                for c in range(0, nk, 2):
                    nc.tensor.matmul(out, a_fn(slice(c, c + 2)), b_fn(slice(c, c + 2)),
                                     start=(c == 0), stop=(c == nk - 2), perf_mode=DR)
            else:
                for c in range(nk):
                    nc.tensor.matmul(out, a_fn(c), b_fn(c),
                                     start=(c == 0), stop=(c == nk - 1))

        def ptile(pool, shape, tg):
            return pool.tile(shape, F32, tag=tg, name="pst_" + tg)

        def ptile_bf(pool, shape, tg):
            return pool.tile(shape, BF16, tag=tg, name="pstb_" + tg)

        # ---- constants. All input DMAs go through the sync queue in
        # priority order (x first, then wv/wq/wk/wo in consumption order) so
        # the front-end is never waiting on a transfer that queued late.
        ident = const.tile([128, 128], BF16)
        make_identity(nc, ident)
        ones_col = const.tile([128, 1], BF16)
        nc.vector.memset(ones_col, 1.0)
        ones_row = const.tile([1, 128], BF16)
        nc.vector.memset(ones_row, 1.0)
        x_sb = const.tile([128, NT, D], BF16)
        xr = ext["x_ext"].rearrange("(t p) d -> p t d", p=128)
        for t in range(NT):
            nc.sync.dma_start(out=x_sb[:, t, :], in_=xr[:, t, :])
        eps_tile = const.tile([128, 1], F32)
        nc.vector.memset(eps_tile, LN_EPS)

        def bcast(name):
            t_ = const.tile([128, D], BF16, tag=f"bc_{name}")
            src = ext[f"{name}_ext"][:]
            ap = bass.AP(tensor=src.tensor, offset=src.offset,
                         ap=[[0, 128]] + list(src.ap))
            nc.gpsimd.dma_start(out=t_, in_=ap)
            return t_

        bv_bc = bcast("bv")
        bo_bc = bcast("bo")
        b2_bc = bcast("b2")
        bq_sb = const.tile([128, 8], F32)
        nc.gpsimd.dma_start(out=bq_sb, in_=ext["bq_ext"].rearrange("(j p) -> p j", p=128))
        bk_sb = const.tile([128, 8], F32)
        nc.gpsimd.dma_start(out=bk_sb, in_=ext["bk_ext"].rearrange("(j p) -> p j", p=128))
        b1_sb = const.tile([128, 32], F32)
        nc.gpsimd.dma_start(out=b1_sb, in_=ext["b1_ext"].rearrange("(j p) -> p j", p=128))
        bias0 = const.tile([128, 512], BF16)
        nc.gpsimd.dma_start(out=bias0, in_=ext["bias0_ext"][:, :])
        biasr = const.tile([128, 512], BF16)
        nc.gpsimd.dma_start(out=biasr, in_=ext["biasr_ext"][:, :])

        x2_sb = resid.tile([128, 4, D], F32)
        h2T = resid.tile([128, 8, SEG], BF16)

        # ---- PE warmup: prime the p-state ramp while x loads (PE must stay
        # busy ~3us continuously to reach full clock, and idle gaps re-throttle
        # it to half clock) ----
        for i in range(24):
            wu = ptile_bf(ptr, [128, 512], "ptr")
            for q4 in range(4):
                nc.tensor.transpose(wu[:, q4 * 128:(q4 + 1) * 128], ident, ident)

        with st(name="attnw", bufs=1) as attnw, st(name="scr", bufs=3) as scr, \
             st(name="soft", bufs=5) as soft:
            wo_sb = attnw.tile([128, 8, D], FP8)
            qT = attnw.tile([128, 8, SEG], BF16)
            kT = attnw.tile([128, 8, T], BF16)
            v_sb = attnw.tile([128, NT, D], BF16)
            ctxT = attnw.tile([128, 8, SEG], FP8)

            with st(name="qkvw", bufs=1) as qkvw:
                wv_sb = qkvw.tile([128, 8, D], FP8)
                nc.gpsimd.dma_start(out=wv_sb, in_=ext["wv_ext"].rearrange("(k p) n -> p k n", p=128))
                wq_sb = qkvw.tile([128, 8, D], FP8)
                nc.gpsimd.dma_start(out=wq_sb, in_=ext["wq_ext"].rearrange("(k p) n -> p k n", p=128))
                wk_sb = qkvw.tile([128, 8, D], FP8)
                nc.gpsimd.dma_start(out=wk_sb, in_=ext["wk_ext"].rearrange("(k p) n -> p k n", p=128))
                nc.gpsimd.dma_start(out=wo_sb, in_=ext["wo_ext"].rearrange("(k p) n -> p k n", p=128))
                hT = qkvw.tile([128, 8, T], FP8)

                # ---- LN1: stats stream in as x tiles land (DVE only, no
                # cross-engine ping-pong), then ONE batched rstd for all 5
                # tiles, then apply/transpose/V per tile.
                mv_all = qkvw.tile([128, NT, 2], F32)
                for t in range(NT):
                    stats = ln.tile([128, 2, 6], F32, tag="ln_stats")
                    xr = x_sb[:, t, :].rearrange("p (s f) -> p s f", f=512)
                    for s in range(2):
                        nc.vector.bn_stats(out=stats[:, s, :], in_=xr[:, s, :])
                    nc.vector.bn_aggr(out=mv_all[:, t, :], in_=stats[:, :, :])
                lnv5 = qkvw.tile([128, NT], F32)
                nc.scalar.activation(out=lnv5, in_=mv_all[:, :, 1], func=AF.Ln,
                                     bias=eps_tile, scale=1.0)
                rstd5 = qkvw.tile([128, NT], F32)
                nc.scalar.activation(out=rstd5, in_=lnv5, func=AF.Exp, bias=0.0, scale=-0.5)
                nmr5 = qkvw.tile([128, NT], F32)
                nc.vector.scalar_tensor_tensor(out=nmr5, in0=mv_all[:, :, 0], scalar=-1.0,
                                               in1=rstd5, op0=mybir.AluOpType.mult,
                                               op1=mybir.AluOpType.mult)
                for t in range(NT):
                    h_t = scr.tile([128, D], BF16, tag="h_t")
                    nc.scalar.activation(out=h_t, in_=x_sb[:, t, :], func=AF.Identity,
                                         bias=nmr5[:, t:t + 1], scale=rstd5[:, t:t + 1])
                    for g in range(2):
                        pt = ptile_bf(ptr, [128, 512], "ptr")
                        for jj in range(4):
                            j = g * 4 + jj
                            nc.tensor.transpose(pt[:, jj * 128:(jj + 1) * 128],
                                                h_t[:, j * 128:(j + 1) * 128], ident)
                        dst = hT[:, g * 4:(g + 1) * 4, t * 128:(t + 1) * 128]
                        if (t * 2 + g) % 2 == 0:
                            nc.vector.tensor_copy(out=dst, in_=pt.rearrange("p (j c) -> p j c", j=4))
                        else:
                            nc.scalar.copy(out=dst, in_=pt.rearrange("p (j c) -> p j c", j=4))
                    # V for this tile: lhsT = hT chunk (pairs under DoubleRow)
                    for n in range(2):
                        pv = ptile(pmm, [128, 512], "mm")
                        pmac(pv, lambda c, t=t: hT[:, c, t * 128:(t + 1) * 128],
                             lambda c, n=n: wv_sb[:, c, n * 512:(n + 1) * 512])
                        nc.vector.tensor_add(v_sb[:, t, n * 512:(n + 1) * 512], pv,
                                             bv_bc[:, n * 512:(n + 1) * 512])

                # residual bias pre-add (off critical path; ordered after LN1 reads)
                for t in range(4):
                    nc.vector.tensor_add(x_sb[:, t + 1, :], x_sb[:, t + 1, :], bo_bc)

                # ---- Q/K projections (DoubleRow fp8) ----
                for j in range(8):
                    pq = ptile(pmm, [128, SEG], "mm")
                    pmac(pq, lambda c, j=j: wq_sb[:, c, j * 128:(j + 1) * 128],
                         lambda c: hT[:, c, HALO:T])
                    nc.scalar.activation(out=qT[:, j, :], in_=pq, func=AF.Identity,
                                         bias=bq_sb[:, j:j + 1], scale=1.0)
                for j in range(8):
                    pk = ptile(pmm, [128, SEG], "mm")
                    pmac(pk, lambda c, j=j: wk_sb[:, c, j * 128:(j + 1) * 128],
                         lambda c: hT[:, c, HALO:T])
                    nc.scalar.activation(out=kT[:, j, HALO:T], in_=pk, func=AF.Identity,
                                         bias=bk_sb[:, j:j + 1], scale=1.0)
                    pkh = ptile(pmm, [128, 512], "mm")
                    pmac(pkh[:, 0:HALO], lambda c, j=j: wk_sb[:, c, j * 128:(j + 1) * 128],
                         lambda c: hT[:, c, 0:HALO])
                    nc.vector.tensor_scalar_add(kT[:, j, 0:HALO], pkh[:, 0:HALO], bk_sb[:, j:j + 1])

            if os.environ.get("K_FRONTONLY", "0") == "1":
                if os.environ.get("K_FRONTPLUS", "0") == "1":
                    ps0 = ptile(pscore, [128, 512], "psc")
                    nc.tensor.matmul(ps0[:, 0:256],
                                     qT[0:64, 0, 0:128], kT[0:64, 0, 0:256],
                                     start=True, stop=True)
                    mode = os.environ.get("K_FRONTPLUS_R64", "0")
                    if mode == "1":
                        nc.tensor.matmul(ps0[:, 256:512],
                                         qT[64:128, 0, 0:128], kT[64:128, 0, 0:256],
                                         start=True, stop=True)
                    elif mode == "off0":
                        ps1 = ptile(pscore, [128, 512], "psc")
                        nc.tensor.matmul(ps1[:, 0:256],
                                         qT[64:128, 0, 0:128], kT[64:128, 0, 0:256],
                                         start=True, stop=True)
                    elif mode == "r0off":
                        nc.tensor.matmul(ps0[:, 256:512],
                                         qT[0:64, 0, 0:128], kT[0:64, 0, 0:256],
                                         start=True, stop=True)
                    if os.environ.get("K_FRONTPLUS_EXP", "0") == "1":
                        pe0 = soft.tile([128, 512], BF16, tag="pE")
                        nc.scalar.activation(out=pe0, in_=ps0, func=AF.Exp, bias=0.0, scale=1.0)
                    else:
                        pe0 = soft.tile([128, 512], BF16, tag="pE")
                        nc.vector.tensor_copy(out=pe0, in_=ps0)
                outr0 = ext["out_ext"].rearrange("(t p) d -> p t d", p=128)
                for t in range(4):
                    o0 = scr.tile([128, D], F32, tag="h_t")
                    nc.vector.tensor_copy(out=o0, in_=v_sb[:, t, :])
                    nc.sync.dma_start(out=outr0[:, t, :], in_=o0)
                return

            # ---- attention. Scores are computed TRANSPOSED (sT[keys, q]):
            # the exp() activation evicts PSUM straight into the probability
            # tile pT (no PE transpose, no separate eviction), per-head key
            # sums come from a ones-column matmul on the PE, and the 1/sum
            # normalization is folded into the ctx PSUM eviction via a
            # PE-broadcast reciprocal tile. Two-slot software pipeline so the
            # PE never waits on the exp/recip chain.
            def s_block(qb, j2):
                """PE: 4 transposed-score matmuls sT[keys,q]; ACT: exp straight
                to SBUF pT; DVE: multiplicative 0/1 window mask (masked exp
                values are garbage but finite — the mask zeroes them before
                the sums/ctx matmuls read pT). Block b = kt*2+hi."""
                mask_t = bias0 if qb == 0 else biasr
                ps = ptile(pmm if os.environ.get("K_SCOREPOOL") == "mm" else pscore,
                           [128, 512], "mm" if os.environ.get("K_SCOREPOOL") == "mm" else "psc")
                if os.environ.get("K_OLDSCORE", "0") == "1":
                    for hi, r in enumerate((0, 64)):
                        nc.tensor.matmul(ps[:, hi * 256:(hi + 1) * 256],
                                         qT[r:r + 64, j2, qb * 128:(qb + 1) * 128],
                                         kT[r:r + 64, j2, qb * 128:qb * 128 + 256],
                                         start=True, stop=True)
                else:
                    for kt in range(2):
                        for hi, r in enumerate((0, 64)):
                            b = kt * 2 + hi
                            nc.tensor.matmul(ps[:, b * 128:(b + 1) * 128],
                                             kT[r:r + 64, j2, (qb + kt) * 128:(qb + kt + 1) * 128],
                                             qT[r:r + 64, j2, qb * 128:(qb + 1) * 128],
                                             start=True, stop=True)
                pE = soft.tile([128, 512], BF16, tag="pE")
                _LVL = int(os.environ.get("K_ATTLVL", "9"))
                if _LVL == 0:
                    nc.vector.tensor_copy(out=pE, in_=ps)
                else:
                    nc.scalar.activation(out=pE, in_=ps, func=AF.Exp, bias=0.0, scale=1.0)
                if _LVL <= 1 and _LVL >= 0 and os.environ.get("K_NOMASK", "0") == "1":
                    return pE
                pT = soft.tile([128, 512], BF16, tag="pT")
                nc.vector.tensor_mul(pT, pE, mask_t)
                return pT

            _HANGTEST = os.environ.get("K_HANGTEST", "0") == "1"

            def sums_block(qb, j2, pT):
                if _HANGTEST:
                    return None, None
                """PE: per-head key-sums (ones-column matmul over both key
                tiles; head sums land at PSUM partitions 0 and 64 — matmul
                outputs must be quadrant-aligned); DVE: reciprocal into a
                [2,128] bf16 row tile."""
                # one PSUM tile bundles ctx (cols 0:128), key-sums (128:256)
                # and the broadcast reciprocal (256:384) -- PSUM pools round
                # every tag up to a full bank, so split tiles would not fit.
                bundle = ptile(pctx, [128, 384], "sb")
                rs = bundle[:, 128:256]
                for hi in range(2):
                    for kt in range(2):
                        b = kt * 2 + hi
                        nc.tensor.matmul(rs[hi * 64:hi * 64 + 1, :], ones_col,
                                         pT[:, b * 128:(b + 1) * 128],
                                         start=(kt == 0), stop=(kt == 1),
                                         tile_position=(0, hi * 64))
                rinv_a = soft.tile([1, 128], BF16, tag="rinva")
                rinv_b = soft.tile([1, 128], BF16, tag="rinvb")
                with nc.allow_low_precision(reason="softmax 1/sum in bf16"):
                    nc.vector.reciprocal(rinv_a, rs[0:1, :])
                    nc.vector.reciprocal(rinv_b, rs[64:65, :])
                return bundle, (rinv_a, rinv_b)

            def ctx_block(qb, j2, pT, bundle, rinv):
                """PE: broadcast rinv to [128,q] + ctx matmuls; DVE: normalize
                during the ctx PSUM eviction."""
                if _HANGTEST:
                    pc = ptile(pscore, [128, 512], "psc")[:, 0:128]
                    for hi, r in enumerate((0, 64)):
                        h = 2 * j2 + hi
                        for kt in range(2):
                            b = kt * 2 + hi
                            nc.tensor.matmul(pc[r:r + 64, :],
                                             v_sb[:, qb + kt, h * 64:(h + 1) * 64],
                                             pT[:, b * 128:(b + 1) * 128],
                                             start=(kt == 0), stop=(kt == 1),
                                             tile_position=(0, r))
                    nc.scalar.copy(out=ctxT[:, j2, qb * 128:(qb + 1) * 128], in_=pc)
                    return
                rinv_a, rinv_b = rinv
                rb = bundle[:, 256:384]
                nc.tensor.matmul(rb[0:64, :], ones_row[:, 0:64], rinv_a,
                                 start=True, stop=True, tile_position=(0, 0))
                nc.tensor.matmul(rb[64:128, :], ones_row[:, 0:64], rinv_b,
                                 start=True, stop=True, tile_position=(0, 64))
                pc = bundle[:, 0:128]
                for hi, r in enumerate((0, 64)):
                    h = 2 * j2 + hi
                    for kt in range(2):
                        b = kt * 2 + hi
                        nc.tensor.matmul(pc[r:r + 64, :],
                                         v_sb[:, qb + kt, h * 64:(h + 1) * 64],
                                         pT[:, b * 128:(b + 1) * 128],
                                         start=(kt == 0), stop=(kt == 1),
                                         tile_position=(0, r))
                # DVE may read only one PSUM operand: stage raw ctx to SBUF
                # on the Scalar engine, then normalize against the PSUM
                # broadcast tile on DVE.
                craw = soft.tile([128, 128], BF16, tag="craw")
                nc.scalar.copy(out=craw, in_=pc)
                nc.vector.tensor_mul(ctxT[:, j2, qb * 128:(qb + 1) * 128], craw, rb)

            def oproj_now(t):
                """Out-projection + residual + LN2 stats (DVE-only start of
                the LN2 chain — the cross-engine pieces are deferred so no
                engine queue-head blocks the attention pipeline)."""
                for n in range(2):
                    po = ptile(pmm, [128, 512], "mm")
                    pmac(po, lambda c, t=t: ctxT[:, c, t * 128:(t + 1) * 128],
                         lambda c, n=n: wo_sb[:, c, n * 512:(n + 1) * 512])
                    sl = slice(n * 512, (n + 1) * 512)
                    nc.vector.tensor_add(x2_sb[:, t, sl], po, x_sb[:, t + 1, sl])
                stats = ln.tile([128, 2, 6], F32, tag="ln_stats")
                xr2 = x2_sb[:, t, :].rearrange("p (s f) -> p s f", f=512)
                for s in range(2):
                    nc.vector.bn_stats(out=stats[:, s, :], in_=xr2[:, s, :])
                mv = ln.tile([128, 2], F32, tag="ln_mv")
                nc.vector.bn_aggr(out=mv[:, :], in_=stats[:, :, :])
                return mv

            def ln2_rstd(t, mv, box):
                lnv = ln.tile([128, 1], F32, tag="ln_lnv")
                nc.scalar.activation(out=lnv, in_=mv[:, 1:2], func=AF.Ln,
                                     bias=eps_tile, scale=1.0)
                rstd = ln.tile([128, 1], F32, tag="ln_rstd")
                nc.scalar.activation(out=rstd, in_=lnv, func=AF.Exp, bias=0.0, scale=-0.5)
                box.append(rstd)

            def ln2_nmr(t, mv, box):
                rstd = box[0]
                nmr = ln.tile([128, 1], F32, tag="ln_nmr")
                nc.vector.scalar_tensor_tensor(out=nmr, in0=mv[:, 0:1], scalar=-1.0,
                                               in1=rstd, op0=mybir.AluOpType.mult,
                                               op1=mybir.AluOpType.mult)
                box.append(nmr)

            def ln2_apply(t, mv, box):
                rstd, nmr = box[0], box[1]
                h2_t = scr.tile([128, D], BF16, tag="h2_t")
                nc.scalar.activation(out=h2_t, in_=x2_sb[:, t, :], func=AF.Identity,
                                     bias=nmr, scale=rstd)
                for g in range(2):
                    pt = ptile_bf(ptr, [128, 512], "ptr")
                    for jj in range(4):
                        j = g * 4 + jj
                        nc.tensor.transpose(pt[:, jj * 128:(jj + 1) * 128],
                                            h2_t[:, j * 128:(j + 1) * 128], ident)
                    dst = h2T[:, g * 4:(g + 1) * 4, t * 128:(t + 1) * 128]
                    if (t * 2 + g) % 2 == 0:
                        nc.vector.tensor_copy(out=dst, in_=pt.rearrange("p (j c) -> p j c", j=4))
                    else:
                        nc.scalar.copy(out=dst, in_=pt.rearrange("p (j c) -> p j c", j=4))
                # final-residual bias pre-add (after LN2 consumed x2[t])
                nc.vector.tensor_add(x2_sb[:, t, :], x2_sb[:, t, :], b2_bc)

            _ATTLVL = int(os.environ.get("K_ATTLVL", "9"))
            if _ATTLVL == 0:
                _ATTLVL = 1
            if _ATTLVL == 1:
                # scores+exp+mask only
                keep = []
                for it in range(int(os.environ.get("K_NITER", "32"))):
                    qb, j2 = divmod(it, 8)
                    keep.append(s_block(qb, j2))
                outr1 = ext["out_ext"].rearrange("(t p) d -> p t d", p=128)
                for t in range(4):
                    o1 = scr.tile([128, D], F32, tag="h_t")
                    nc.vector.tensor_copy(out=o1[:, 0:512], in_=keep[-1])
                    nc.vector.tensor_copy(out=o1[:, 512:1024], in_=keep[len(keep) >= 2 and -2 or -1])
                    nc.sync.dma_start(out=outr1[:, t, :], in_=o1)
                return

            pend1 = []   # awaiting sums (lag 1)
            pend2 = []   # awaiting ctx (lag 2)
            deferred = []
            for it in range(32 + 2):
                if it < 32:
                    qb, j2 = divmod(it, 8)
                    pend1.append((qb, j2, s_block(qb, j2)))
                if pend1 and (len(pend1) > 1 or it >= 32):
                    qb1, j21, pT1 = pend1.pop(0)
                    bundle1, rinv1 = sums_block(qb1, j21, pT1)
                    pend2.append((qb1, j21, pT1, bundle1, rinv1))
                if pend2 and (len(pend2) > 1 or it >= 32):
                    qb2, j22, pT2, bundle2, rinv2 = pend2.pop(0)
                    ctx_block(qb2, j22, pT2, bundle2, rinv2)
                    if j22 == 7:
                        mv = oproj_now(qb2)
                        box = []
                        deferred.extend([
                            lambda t=qb2, mv=mv, box=box: ln2_rstd(t, mv, box),
                            lambda t=qb2, mv=mv, box=box: ln2_nmr(t, mv, box),
                            lambda t=qb2, mv=mv, box=box: ln2_apply(t, mv, box),
                        ])
                    elif deferred:
                        deferred.pop(0)()
            while deferred:
                deferred.pop(0)()

        # ---- FFN (bf16) ----
        with st(name="ffnw", bufs=1) as ffnw, st(name="w1p", bufs=2) as w1p, \
             st(name="outp", bufs=2) as outp:
            gT = ffnw.tile([128, 32, SEG], BF16)
            w2_sb = ffnw.tile([128, 32, D], BF16)

            w2r = ext["w2_ext"].rearrange("(c p) n -> p c n", p=128)
            for c in range(4):
                nc.gpsimd.dma_start(out=w2_sb[:, c * 8:(c + 1) * 8, :],
                                    in_=w2r[:, c * 8:(c + 1) * 8, :])

            w1r = ext["w1_ext"].rearrange("(k p) n -> p k n", p=128)
            for c in range(4):
                w1c = w1p.tile([128, 8, 1024], BF16, tag="w1c")
                nc.gpsimd.dma_start(out=w1c, in_=w1r[:, :, c * 1024:(c + 1) * 1024])
                for jj in range(8):
                    jdff = c * 8 + jj
                    pg = ptile(pmm, [128, SEG], "mm")
                    for k in range(8):
                        nc.tensor.matmul(pg, w1c[:, k, jj * 128:(jj + 1) * 128],
                                         h2T[:, k, :], start=(k == 0), stop=(k == 7))
                    nc.scalar.activation(out=gT[:, jdff, :], in_=pg, func=AF.Gelu_apprx_tanh,
                                         bias=b1_sb[:, jdff:jdff + 1], scale=1.0)

            outr = ext["out_ext"].rearrange("(t p) d -> p t d", p=128)
            for t in range(4):
                o_t = outp.tile([128, D], F32, tag="o_t")
                for n in range(2):
                    py = ptile(pmm, [128, 512], "mm")
                    for k in range(32):
                        nc.tensor.matmul(py, gT[:, k, t * 128:(t + 1) * 128],
                                         w2_sb[:, k, n * 512:(n + 1) * 512],
                                         start=(k == 0), stop=(k == 31))
                    sl = slice(n * 512, (n + 1) * 512)
                    nc.vector.tensor_add(o_t[:, sl], py, x2_sb[:, t, sl])
                nc.sync.dma_start(out=outr[:, t, :], in_=o_t)


def _host_prep(x, Wq, bq, Wk, bk, Wv, bv, Wo, bo, W1, b1, W2, b2,
               ln1_w, ln1_b, ln2_w, ln2_b):
    bf = ml_dtypes.bfloat16
    f8 = ml_dtypes.float8_e4m3fn if _USE_FP8 else bf

    def q8(a):
        return np.ascontiguousarray(
            np.clip(np.asarray(a, np.float32), -240.0, 240.0).astype(f8))

    sc = 1.0 / np.sqrt(DH)
    wq_eff = q8((ln1_w[:, None] * Wq) * sc)
    bq_eff = ((bq + ln1_b @ Wq) * sc).astype(np.float32)
    wk_eff = q8(ln1_w[:, None] * Wk)
    bk_eff = (bk + ln1_b @ Wk).astype(np.float32)
    wv_eff = q8(ln1_w[:, None] * Wv)
    bv_eff = (bv + ln1_b @ Wv).astype(np.float32)
    w1_eff = (ln2_w[:, None] * W1).astype(bf)
    b1_eff = (b1 + ln2_b @ W1).astype(np.float32)

    # Transposed masks for the sT-layout scores: block b = kt*2 + hi holds
    # sT[keys(tile kt), q] for head hi, so the mask tile is the transpose of
    # the [q, keys] mask, duplicated for the two heads of a pair.
    r = np.arange(128)[:, None]
    c = np.arange(128)[None, :]
    left = np.where(c >= r, 1.0, 0.0).astype(np.float32)
    diag = np.where(c <= r, 1.0, 0.0).astype(np.float32)
    fullzero = np.zeros((128, 128), np.float32)
    m0T, m1T = np.ascontiguousarray(left.T), np.ascontiguousarray(diag.T)
    biasr = np.concatenate([m0T, m0T, m1T, m1T], axis=1).astype(bf)
    bias0_halo = np.concatenate([fullzero, fullzero, m1T, m1T], axis=1).astype(bf)

    shared = {
        "wq": wq_eff, "wk": wk_eff, "wv": wv_eff,
        "wo": q8(Wo),
        "w1": w1_eff, "w2": np.ascontiguousarray(W2.astype(bf)),
        "bq": bq_eff, "bk": bk_eff, "bv": bv_eff.astype(bf),
        "bo": bo.astype(bf), "b1": b1_eff, "b2": b2.astype(bf),
        "biasr": biasr,
    }
    in_maps = []
    for core in range(NSEG):
        b_, s_ = core // 4, core % 4
        if s_ == 0:
            seg = np.concatenate(
                [np.zeros((HALO, D), np.float32), x[b_, 0:SEG]], axis=0)
            bias0 = bias0_halo
        else:
            seg = x[b_, s_ * SEG - HALO: (s_ + 1) * SEG]
            bias0 = biasr
        m = dict(shared)
        m["x"] = np.ascontiguousarray(seg.astype(bf))
        m["bias0"] = bias0
        in_maps.append(m)
    return in_maps


def kernel(**inputs):
    from concourse.bass_utils import run_bass_kernel_spmd

    if "nc" not in _CACHED:
        _CACHED["nc"] = _build()
    nc = _CACHED["nc"]

    in_maps = _host_prep(**{k: np.asarray(v) for k, v in inputs.items()})
    trace = bool(int(os.environ.get("KERNEL_TRACE", "0")))
    res = run_bass_kernel_spmd(nc, in_maps, list(range(NSEG)), trace=trace)
    kernel.last_results = res

    x = np.asarray(inputs["x"])
    out = np.empty((B, L, D), np.float32)
    for core in range(NSEG):
        b_, s_ = core // 4, core % 4
        out[b_, s_ * SEG:(s_ + 1) * SEG] = res.results[core]["out"]
    return out
